# revision 15
# baseline (speedup 1.0000x reference)
"""BiGRU encoder kernel for 8 Trainium2 NeuronCores (fp16, exact ragged schedule).

Strategy:
  - Masked fixed-position reformulation: forward runs positions ascending into
    the center, backward descending into the center, so every sample's
    recurrence ENDS on the final step.  A sample of length l starts updating
    at the step where need == l; before that its hidden state is held at 0
    exactly by forcing z = 1 (+BIG on the z pre-activation).
  - Samples sorted by window_len, dealt round-robin to 8 cores (data
    parallel).  Each core holds ONE sorted batch of 1024 columns
    (features on SBUF partitions, samples on the free dim).  Step j runs on
    the exact suffix W_j = max over cores of #samples with len >= need --
    fp16 matmuls are full rate at any width, so no minimum-width padding.
  - Within a step, columns split into F (samples taking their first step:
    h == 0) and R (already running).  The hidden projection runs ONLY on R;
    F columns take a cheap h'=(1-z)n update that never reads h_prev.
    Cross-core width slack is fixed up by a narrow mask strip on z.
  - The suffix splits into 512-wide groups (PSUM bank limit).  Gate biases
    are folded into scalar_tensor_tensor ops so tanh and the h-update run as
    single wide ops over [128, 4, W].
  - Output is written feature-major (yT) and transposed on the host.
"""

import os
from contextlib import ExitStack

import numpy as np

import concourse.bacc as bacc
import concourse.tile as tile
from concourse import mybir
from concourse.bass_utils import run_bass_kernel_spmd

NCORES = 8
B, T, D, H = 8192, 15, 512, 512
G = 3 * H
Bc = B // NCORES  # 1024 columns per core
BIG = 40.0
F32 = mybir.dt.float32
DT = mybir.dt.float16

ACT = mybir.ActivationFunctionType
ALU = mybir.AluOpType

_PROGRAM_CACHE = {}
LAST_RESULT = None


def _ceil(a, b):
    return -(-a // b)


def _build_program(sched):
    """sched = (f_steps, b_steps); each steps = tuple of (W, strip) per step,
    W monotone nondecreasing, W[-1] == Bc."""
    f_steps, b_steps = sched
    nf, nb = len(f_steps), len(b_steps)

    nc = bacc.Bacc("TRN2", target_bir_lowering=False, debug=False,
                   num_devices=NCORES)

    S4 = 4 * (sum(w for w, _ in f_steps) + sum(w for w, _ in b_steps))
    xpk_d = nc.dram_tensor("xpk", [128, S4], DT, kind="ExternalInput")
    wf_d = nc.dram_tensor("wf", [D + H, G], DT, kind="ExternalInput")
    wb_d = nc.dram_tensor("wb", [D + H, G], DT, kind="ExternalInput")
    w1_d = nc.dram_tensor("w1", [2 * H, H], DT, kind="ExternalInput")
    w2_d = nc.dram_tensor("w2", [H, H], DT, kind="ExternalInput")
    bias_d = nc.dram_tensor("bias", [40, 128], F32, kind="ExternalInput")
    mf_d = nc.dram_tensor("maskzf", [nf, Bc], F32, kind="ExternalInput")
    mb_d = nc.dram_tensor("maskzb", [nb, Bc], F32, kind="ExternalInput")
    y_d = nc.dram_tensor("yT", [H, Bc], F32, kind="ExternalOutput")

    with tile.TileContext(nc) as tc, ExitStack() as ctx:
        const = ctx.enter_context(tc.tile_pool(name="const", bufs=1))
        wpool = ctx.enter_context(tc.tile_pool(name="w", bufs=2))
        xfp = ctx.enter_context(tc.tile_pool(name="xf", bufs=3))
        xbp = ctx.enter_context(tc.tile_pool(name="xb", bufs=2))
        hpool = ctx.enter_context(tc.tile_pool(name="h", bufs=2))
        hfin = ctx.enter_context(tc.tile_pool(name="hfin", bufs=2))
        rz4p = ctx.enter_context(tc.tile_pool(name="rz4", bufs=2))
        ssp = ctx.enter_context(tc.tile_pool(name="ss", bufs=2))
        np_ = ctx.enter_context(tc.tile_pool(name="n4", bufs=2))
        scr = ctx.enter_context(tc.tile_pool(name="scr", bufs=2))
        ttp = ctx.enter_context(tc.tile_pool(name="tt", bufs=4))
        obuf = ctx.enter_context(tc.tile_pool(name="o", bufs=2))
        mpool = ctx.enter_context(tc.tile_pool(name="m", bufs=2))
        accp = ctx.enter_context(tc.tile_pool(name="mlp", bufs=2))
        rzps = ctx.enter_context(tc.tile_pool(name="rz", bufs=2, space="PSUM"))
        xpps = ctx.enter_context(tc.tile_pool(name="xp", bufs=2, space="PSUM"))
        ghps = ctx.enter_context(tc.tile_pool(name="gh", bufs=2, space="PSUM"))

        # Weights as [128, kchunk, gate-cols]; kchunks 0-3 input dims, 4-7
        # hidden dims.  Per-kchunk DMAs so the first matmuls start as soon
        # as chunk 0 lands.
        def load_w(dram, kchunks, cols, name, pool, tag, eng):
            t_ = pool.tile([128, kchunks, cols], DT, tag=tag, name=name)
            src = dram.rearrange("(c k) g -> k c g", k=128)
            for c in range(kchunks):
                eng.dma_start(t_[:, c, :], src[:, c, :])
            return t_

        bt = const.tile([128, 40], F32)
        nc.gpsimd.dma_start(bt[:], bias_d.rearrange("n p -> p n"))
        wf = load_w(wf_d, 8, G, "wf", wpool, "w", nc.scalar)
        wb = load_w(wb_d, 8, G, "wb", wpool, "w", nc.gpsimd)
        w1 = load_w(w1_d, 8, H, "w1", const, "w1", nc.gpsimd)
        w2 = load_w(w2_d, 4, H, "w2", const, "w2", nc.gpsimd)

        def emit_x(steps, pool, tag, j, off):
            # x tile holds the step's suffix packed [4k x W] contiguously per
            # partition -- single fat DMA run on both sides.
            W = steps[j][0]
            xt = pool.tile([128, 4 * Bc], DT, tag=tag, name=f"{tag}{j}")
            nc.sync.dma_start(xt[:, :4 * W], xpk_d[:, off:off + 4 * W])
            return xt

        def emit_step(j, steps, xt, h_prev, w, mask_d, bb, is_last):
            """One GRU step.  Local cols 0..W-1 map to global Bc-W..Bc-1.
            F = [0, Fw): first-step columns.  R = [Fw, W): running."""
            W, strip = steps[j]
            Wprev = steps[j - 1][0] if j > 0 else 0
            Fw = W - Wprev
            goff = Bc - W  # local -> global

            h_next = (hfin if is_last else hpool).tile(
                [128, 4, Bc], DT, tag="hfin" if is_last else "h", name="h")

            mt = None
            if strip > 0:
                mt = mpool.tile([128, 256], F32, tag="m", name="mt")
                nc.sync.dma_start(
                    mt[:, :strip],
                    mask_d[j, goff:goff + strip].partition_broadcast(128),
                )

            ngroups = _ceil(W, 512)
            for g in range(ngroups):  # left-aligned groups on local coords
                glo, ghi = 512 * g, min(512 * (g + 1), W)
                gw = ghi - glo
                fl, fh = glo, max(min(ghi, Fw), glo)   # F within group
                rl, rh = max(glo, Fw), ghi             # R within group
                fw, rw = fh - fl, rh - rl
                sl, sh = glo, max(min(ghi, strip), glo)  # mask strip in group
                sw = sh - sl

                rz = []
                xpn = []
                ghn = []
                for i in range(4):
                    ro, zo, no = i * 128, H + i * 128, 2 * H + i * 128
                    rzt = rzps.tile([128, 2, 512], F32, tag="rz", name=f"rz{i}")
                    xpt = xpps.tile([128, 512], F32, tag="xp", name=f"xp{i}")
                    rz.append(rzt)
                    xpn.append(xpt)
                    for k in range(4):
                        st = k == 0
                        xk = xt[:, k * W + glo:k * W + ghi]
                        if fw > 0 and rw > 0:
                            # F: start opens the bank (lazy-zeroes it); the
                            # R x-proj piggybacks with start=False and gets
                            # zero-init from the pending-zero region.  The
                            # h-proj's stop closes the bank group.
                            xkF = xt[:, k * W + fl:k * W + fh]
                            xkR = xt[:, k * W + rl:k * W + rh]
                            nc.tensor.matmul(rzt[:, 0, :fw], w[:, k, ro:ro + 128],
                                             xkF, start=st, stop=False)
                            nc.tensor.matmul(rzt[:, 0, fw:gw], w[:, k, ro:ro + 128],
                                             xkR, start=False, stop=False,
                                             skip_group_check=True)
                            nc.tensor.matmul(rzt[:, 1, :fw], w[:, k, zo:zo + 128],
                                             xkF, start=st, stop=False)
                            nc.tensor.matmul(rzt[:, 1, fw:gw], w[:, k, zo:zo + 128],
                                             xkR, start=False, stop=False,
                                             skip_group_check=True)
                        else:
                            sp = k == 3 and rw == 0
                            nc.tensor.matmul(rzt[:, 0, :gw], w[:, k, ro:ro + 128],
                                             xk, start=st, stop=sp)
                            nc.tensor.matmul(rzt[:, 1, :gw], w[:, k, zo:zo + 128],
                                             xk, start=st, stop=sp)
                        nc.tensor.matmul(xpt[:, :gw], w[:, k, no:no + 128],
                                         xk, start=st, stop=k == 3)
                    if rw > 0:
                        ght = ghps.tile([128, 512], F32, tag="gh", name=f"gh{i}")
                        ghn.append(ght)
                        fo = rl - glo  # local-in-group offset of R
                        for k in range(4):
                            hk = h_prev[:, k, goff + rl:goff + rh]
                            nc.tensor.matmul(rzt[:, 0, fo:gw],
                                             w[:, 4 + k, ro:ro + 128], hk,
                                             start=False, stop=k == 3)
                            nc.tensor.matmul(rzt[:, 1, fo:gw],
                                             w[:, 4 + k, zo:zo + 128], hk,
                                             start=False, stop=k == 3)
                            nc.tensor.matmul(ght[:, :rw],
                                             w[:, 4 + k, no:no + 128], hk,
                                             start=k == 0, stop=k == 3)

                # --- elementwise chain for this group, per i-pair so the
                # next step's h-projection k-chunks can start early ---
                rz4 = rz4p.tile([128, 2, 4, 512], DT, tag="rz4", name="rz4")
                ss4 = ssp.tile([128, 4, 512], DT, tag="ss", name="ss4")
                n4 = np_.tile([128, 4, 512], DT, tag="n4", name="n4")
                sc = scr.tile([128, 4, 512], DT, tag="scr", name="sc")

                def upd_pair(p0):
                    ii = slice(p0, p0 + 2)
                    nc.scalar.activation(n4[:, ii, :gw], ss4[:, ii, :gw],
                                         ACT.Tanh)
                    if fw > 0:
                        zF = rz4[:, 1, ii, fl - glo:fh - glo]
                        nF = n4[:, ii, fl - glo:fh - glo]
                        eF = sc[:, ii, fl - glo:fh - glo]
                        nc.vector.tensor_mul(eF, zF, nF)
                        nc.vector.tensor_sub(h_next[:, ii, goff + fl:goff + fh],
                                             nF, eF)
                    if rw > 0:
                        zR = rz4[:, 1, ii, rl - glo:rh - glo]
                        nR = n4[:, ii, rl - glo:rh - glo]
                        dd = sc[:, ii, rl - glo:rh - glo]
                        nc.vector.tensor_sub(dd, h_prev[:, ii, goff + rl:goff + rh],
                                             nR)
                        nc.vector.tensor_mul(dd, zR, dd)
                        nc.vector.tensor_add(h_next[:, ii, goff + rl:goff + rh],
                                             nR, dd)

                for i in range(4):
                    # r = sigmoid(rps + bias_r)
                    nc.scalar.activation(rz4[:, 0, i, :gw], rz[i][:, 0, :gw],
                                         ACT.Sigmoid, bias=bt[:, bb + i:bb + i + 1])
                    # mask strip: force z -> 1 on over-included columns
                    if sw > 0:
                        nc.vector.tensor_add(rz[i][:, 1, sl - glo:sh - glo],
                                             rz[i][:, 1, sl - glo:sh - glo],
                                             mt[:, sl:sh])
                    nc.scalar.activation(rz4[:, 1, i, :gw], rz[i][:, 1, :gw],
                                         ACT.Sigmoid,
                                         bias=bt[:, bb + 4 + i:bb + 5 + i])
                    # n pre-activation: ss = xpn + bih_n + r * (ghn + bhh_n)
                    if fw > 0:
                        t1 = ttp.tile([128, 512], DT, tag="tt", name="t1")
                        nc.vector.tensor_scalar(
                            t1[:, :fw], rz4[:, 0, i, fl - glo:fh - glo],
                            bt[:, bb + 8 + i:bb + 9 + i],
                            bt[:, bb + 12 + i:bb + 13 + i],
                            op0=ALU.mult, op1=ALU.add)
                        nc.vector.tensor_add(
                            ss4[:, i, fl - glo:fh - glo], t1[:, :fw],
                            xpn[i][:, fl - glo:fh - glo])
                    if rw > 0:
                        t2 = ttp.tile([128, 512], DT, tag="tt", name="t2")
                        nc.vector.scalar_tensor_tensor(
                            t2[:, :rw], ghn[i][:, :rw],
                            bt[:, bb + 8 + i:bb + 9 + i],
                            rz4[:, 0, i, rl - glo:rh - glo],
                            op0=ALU.add, op1=ALU.mult)
                        nc.vector.scalar_tensor_tensor(
                            ss4[:, i, rl - glo:rh - glo], t2[:, :rw],
                            bt[:, bb + 12 + i:bb + 13 + i],
                            xpn[i][:, rl - glo:rh - glo],
                            op0=ALU.add, op1=ALU.add)
                    if i == 1:
                        upd_pair(0)
                    elif i == 3:
                        upd_pair(2)
            return h_next

        def emit_dir(steps, w, mask_d, bb, pool, tag, off0):
            n = len(steps)
            h = None
            off = off0
            for j in range(n):
                xt = emit_x(steps, pool, tag, j, off)
                off += 4 * steps[j][0]
                h = emit_step(j, steps, xt, h, w, mask_d, bb, j == n - 1)
            return h

        hf4 = emit_dir(f_steps, wf, mf_d, 0, xfp, "xf", 0)

        # MLP phase A: acc = W1[:, :H].T @ hf  (runs while backward GRU owns
        # the critical path; result parked in SBUF)
        acc = accp.tile([128, 4, Bc], DT, tag="mlp", name="acc")
        for g in range(Bc // 512):
            for i in range(4):
                ps = xpps.tile([128, 512], F32, tag="xp", name="mlpA")
                for k in range(4):
                    nc.tensor.matmul(ps[:], w1[:, k, i * 128:(i + 1) * 128],
                                     hf4[:, k, g * 512:(g + 1) * 512],
                                     start=k == 0, stop=k == 3)
                nc.scalar.activation(acc[:, i, g * 512:(g + 1) * 512], ps[:],
                                     ACT.Copy)

        hb4 = emit_dir(b_steps, wb, mb_d, 16, xbp, "xb",
                       4 * sum(w for w, _ in f_steps))

        # MLP phases B+C interleaved per column group:
        #   hid = relu(acc + W1[:, H:].T @ hb + b1);  y = W2.T @ hid + b2
        hid = accp.tile([128, 4, Bc], DT, tag="mlp", name="hid")
        for g in range(Bc // 512):
            gs = slice(g * 512, (g + 1) * 512)
            pre = ssp.tile([128, 4, 512], DT, tag="ss", name="pre")
            for i in range(4):
                ps = xpps.tile([128, 512], F32, tag="xp", name="mlpB")
                for k in range(4):
                    nc.tensor.matmul(ps[:], w1[:, 4 + k, i * 128:(i + 1) * 128],
                                     hb4[:, k, gs], start=k == 0, stop=k == 3)
                nc.vector.scalar_tensor_tensor(
                    pre[:, i, :], ps[:], bt[:, 32 + i:33 + i],
                    acc[:, i, gs], op0=ALU.add, op1=ALU.add)
            nc.scalar.activation(hid[:, :, gs], pre[:], ACT.Relu)
            for i in range(4):
                ps = xpps.tile([128, 512], F32, tag="xp", name="mlpC")
                for k in range(4):
                    nc.tensor.matmul(ps[:], w2[:, k, i * 128:(i + 1) * 128],
                                     hid[:, k, gs], start=k == 0, stop=k == 3)
                o32 = obuf.tile([128, 512], F32, tag="o", name="o32")
                nc.scalar.activation(o32[:], ps[:], ACT.Identity,
                                     bias=bt[:, 36 + i:37 + i])
                nc.scalar.dma_start(
                    y_d[i * 128:(i + 1) * 128, gs], o32[:])

    nc.compile()
    return nc


def kernel(padded_window, window_len, Wih_f, Whh_f, bih_f, bhh_f,
           Wih_b, Whh_b, bih_b, bhh_b, W1, b1, W2, b2):
    wl = np.asarray(window_len)
    lf = (wl - 1) // 2 + 1
    lb = wl // 2 + 1
    order = np.argsort(wl, kind="stable")

    # per-core sorted lengths: row k = per-core rank k, column = core
    lf_pc = lf[order].reshape(-1, NCORES)
    lb_pc = lb[order].reshape(-1, NCORES)

    def dir_steps(lens_pc):
        n = int(lens_pc.max())
        steps, cnts = [], []
        for j in range(n):
            need = n - j
            cnt = (lens_pc >= need).sum(axis=0)  # per core
            W = int(cnt.max())
            strip = W - int(cnt.min())
            assert strip <= 256, f"mask strip {strip} exceeds tile"
            steps.append((W, strip))
            cnts.append(cnt)
        return tuple(steps), cnts

    f_steps, f_cnts = dir_steps(lf_pc)
    b_steps, b_cnts = dir_steps(lb_pc)
    sched = (f_steps, b_steps)

    if sched not in _PROGRAM_CACHE:
        _PROGRAM_CACHE[sched] = _build_program(sched)
    nc = _PROGRAM_CACHE[sched]

    f16 = np.float16
    f32 = np.float32
    wf = np.concatenate([Wih_f.T, Whh_f.T], 0).astype(f16)
    wb = np.concatenate([Wih_b.T, Whh_b.T], 0).astype(f16)
    w1 = np.ascontiguousarray(W1.T, dtype=f16)
    w2 = np.ascontiguousarray(W2.T, dtype=f16)

    def chunks(v):  # [512] -> [4, 128]
        return np.asarray(v, f32).reshape(4, 128)

    bias = np.concatenate([
        chunks((bih_f + bhh_f)[:H]), chunks((bih_f + bhh_f)[H:2 * H]),
        chunks(bhh_f[2 * H:]), chunks(bih_f[2 * H:]),
        chunks((bih_b + bhh_b)[:H]), chunks((bih_b + bhh_b)[H:2 * H]),
        chunks(bhh_b[2 * H:]), chunks(bih_b[2 * H:]),
        chunks(b1), chunks(b2),
    ], 0)  # [40, 128]

    pw16 = np.asarray(padded_window).astype(f16)
    pos = np.arange(Bc)
    nf, nb = len(f_steps), len(b_steps)
    in_maps = []
    for c in range(NCORES):
        idx = order[c::NCORES]
        xTc = pw16[idx].transpose(1, 2, 0)  # [15, 512, Bc] (view-ish)
        blocks = []
        for steps, pfn in ((f_steps, lambda j: 8 - nf + j),
                           (b_steps, lambda j: 6 + nb - j)):
            for j, (W, _) in enumerate(steps):
                sl = xTc[pfn(j), :, Bc - W:]  # [512, W]
                blocks.append(sl.reshape(4, 128, W).transpose(1, 0, 2)
                              .reshape(128, 4 * W))
        xpk = np.ascontiguousarray(np.concatenate(blocks, axis=1))
        mzf = np.stack([(BIG * (pos < Bc - cnt[c])).astype(f32)
                        for cnt in f_cnts])
        mzb = np.stack([(BIG * (pos < Bc - cnt[c])).astype(f32)
                        for cnt in b_cnts])
        in_maps.append({
            "xpk": xpk, "wf": wf, "wb": wb, "w1": w1, "w2": w2,
            "bias": bias, "maskzf": mzf, "maskzb": mzb,
        })

    trace = bool(os.environ.get("GRU_TRACE"))
    kw = {}
    if os.environ.get("GRU_TMPDIR"):
        kw["tmpdir"] = os.environ["GRU_TMPDIR"]
    res = run_bass_kernel_spmd(nc, in_maps, core_ids=list(range(NCORES)),
                               trace=trace, **kw)
    global LAST_RESULT
    LAST_RESULT = res
    out = np.empty((B, H), f32)
    for c in range(NCORES):
        out[order[c::NCORES]] = res.results[c]["yT"].T
    return out


# revision 18
# speedup vs baseline: 1.0093x; 1.0093x over previous
"""BiGRU encoder kernel for 8 Trainium2 NeuronCores (fp16, exact ragged schedule).

Strategy:
  - Masked fixed-position reformulation: forward runs positions ascending into
    the center, backward descending into the center, so every sample's
    recurrence ENDS on the final step.  A sample of length l starts updating
    at the step where need == l; before that its hidden state is held at 0
    exactly by forcing z = 1 (+BIG on the z pre-activation).
  - Samples sorted by window_len, dealt round-robin to 8 cores (data
    parallel).  Each core holds ONE sorted batch of 1024 columns
    (features on SBUF partitions, samples on the free dim).  Step j runs on
    the exact suffix W_j = max over cores of #samples with len >= need --
    fp16 matmuls are full rate at any width, so no minimum-width padding.
  - Within a step, columns split into F (samples taking their first step:
    h == 0) and R (already running).  The hidden projection runs ONLY on R;
    F columns take a cheap h'=(1-z)n update that never reads h_prev.
    Cross-core width slack is fixed up by a narrow mask strip on z.
  - The suffix splits into 512-wide groups (PSUM bank limit).  Gate biases
    are folded into scalar_tensor_tensor ops so tanh and the h-update run as
    single wide ops over [128, 4, W].
  - Output is written feature-major (yT) and transposed on the host.
"""

import os
from contextlib import ExitStack

import numpy as np

import concourse.bacc as bacc
import concourse.tile as tile
from concourse import mybir
from concourse.bass_utils import run_bass_kernel_spmd

NCORES = 8
B, T, D, H = 8192, 15, 512, 512
G = 3 * H
Bc = B // NCORES  # 1024 columns per core
BIG = 40.0
F32 = mybir.dt.float32
DT = mybir.dt.float16

ACT = mybir.ActivationFunctionType
ALU = mybir.AluOpType

_PROGRAM_CACHE = {}
LAST_RESULT = None


def _ceil(a, b):
    return -(-a // b)


def _build_program(sched):
    """sched = (f_steps, b_steps); each steps = tuple of (W, strip) per step,
    W monotone nondecreasing, W[-1] == Bc."""
    f_steps, b_steps = sched
    nf, nb = len(f_steps), len(b_steps)

    nc = bacc.Bacc("TRN2", target_bir_lowering=False, debug=False,
                   num_devices=NCORES)

    S4 = 4 * (sum(w for w, _ in f_steps) + sum(w for w, _ in b_steps))
    xpk_d = nc.dram_tensor("xpk", [128, S4], DT, kind="ExternalInput")
    wf_d = nc.dram_tensor("wf", [D + H, G], DT, kind="ExternalInput")
    wb_d = nc.dram_tensor("wb", [D + H, G], DT, kind="ExternalInput")
    w1_d = nc.dram_tensor("w1", [2 * H, H], DT, kind="ExternalInput")
    w2_d = nc.dram_tensor("w2", [H, H], DT, kind="ExternalInput")
    bias_d = nc.dram_tensor("bias", [40, 128], F32, kind="ExternalInput")
    mf_d = nc.dram_tensor("maskzf", [nf, Bc], F32, kind="ExternalInput")
    mb_d = nc.dram_tensor("maskzb", [nb, Bc], F32, kind="ExternalInput")
    y_d = nc.dram_tensor("yT", [H, Bc], F32, kind="ExternalOutput")

    with tile.TileContext(nc) as tc, ExitStack() as ctx:
        const = ctx.enter_context(tc.tile_pool(name="const", bufs=1))
        wpool = ctx.enter_context(tc.tile_pool(name="w", bufs=2))
        xfp = ctx.enter_context(tc.tile_pool(name="xf", bufs=2))
        xbp = ctx.enter_context(tc.tile_pool(name="xb", bufs=2))
        hfp = ctx.enter_context(tc.tile_pool(name="hf", bufs=2))
        hbp = ctx.enter_context(tc.tile_pool(name="hb", bufs=2))
        hfin = ctx.enter_context(tc.tile_pool(name="hfin", bufs=2))
        rz4p = ctx.enter_context(tc.tile_pool(name="rz4", bufs=2))
        ssp = ctx.enter_context(tc.tile_pool(name="ss", bufs=2))
        np_ = ctx.enter_context(tc.tile_pool(name="n4", bufs=2))
        scr = ctx.enter_context(tc.tile_pool(name="scr", bufs=2))
        ttp = ctx.enter_context(tc.tile_pool(name="tt", bufs=4))
        obuf = ctx.enter_context(tc.tile_pool(name="o", bufs=2))
        mpool = ctx.enter_context(tc.tile_pool(name="m", bufs=2))
        accp = ctx.enter_context(tc.tile_pool(name="mlp", bufs=2))
        rzps = ctx.enter_context(tc.tile_pool(name="rz", bufs=2, space="PSUM"))
        xpps = ctx.enter_context(tc.tile_pool(name="xp", bufs=2, space="PSUM"))
        ghps = ctx.enter_context(tc.tile_pool(name="gh", bufs=2, space="PSUM"))

        # Weights as [128, kchunk, gate-cols]; kchunks 0-3 input dims, 4-7
        # hidden dims.  Per-kchunk DMAs so the first matmuls start as soon
        # as chunk 0 lands.
        def load_w(dram, kchunks, cols, name, pool, tag, eng):
            t_ = pool.tile([128, kchunks, cols], DT, tag=tag, name=name)
            src = dram.rearrange("(c k) g -> k c g", k=128)
            for c in range(kchunks):
                eng.dma_start(t_[:, c, :], src[:, c, :])
            return t_

        bt = const.tile([128, 40], F32)
        nc.gpsimd.dma_start(bt[:], bias_d.rearrange("n p -> p n"))
        wf = load_w(wf_d, 8, G, "wf", wpool, "w", nc.scalar)
        wb = load_w(wb_d, 8, G, "wb", wpool, "w", nc.gpsimd)
        w1 = load_w(w1_d, 8, H, "w1", const, "w1", nc.gpsimd)
        w2 = load_w(w2_d, 4, H, "w2", const, "w2", nc.gpsimd)

        def emit_x(steps, pool, tag, j, off):
            # x tile holds the step's suffix packed [4k x W] contiguously per
            # partition -- single fat DMA run on both sides.
            W = steps[j][0]
            xt = pool.tile([128, 4 * Bc], DT, tag=tag, name=f"{tag}{j}")
            nc.sync.dma_start(xt[:, :4 * W], xpk_d[:, off:off + 4 * W])
            return xt

        def emit_step(j, steps, xt, h_prev, w, mask_d, bb, hpool, tag, is_last):
            """One GRU step.  Local cols 0..W-1 map to global Bc-W..Bc-1.
            F = [0, Fw): first-step columns.  R = [Fw, W): running."""
            W, strip = steps[j]
            Wprev = steps[j - 1][0] if j > 0 else 0
            Fw = W - Wprev
            goff = Bc - W  # local -> global

            h_next = (hfin if is_last else hpool).tile(
                [128, 4, Bc], DT, tag="hfin" if is_last else tag, name="h")

            mt = None
            if strip > 0:
                mt = mpool.tile([128, 256], F32, tag="m", name="mt")
                nc.sync.dma_start(
                    mt[:, :strip],
                    mask_d[j, goff:goff + strip].partition_broadcast(128),
                )

            ngroups = _ceil(W, 512)
            for g in range(ngroups):  # left-aligned groups on local coords
                glo, ghi = 512 * g, min(512 * (g + 1), W)
                gw = ghi - glo
                fl, fh = glo, max(min(ghi, Fw), glo)   # F within group
                rl, rh = max(glo, Fw), ghi             # R within group
                fw, rw = fh - fl, rh - rl
                sl, sh = glo, max(min(ghi, strip), glo)  # mask strip in group
                sw = sh - sl

                rz = []
                xpn = []
                ghn = []
                for i in range(4):
                    ro, zo, no = i * 128, H + i * 128, 2 * H + i * 128
                    rzt = rzps.tile([128, 2, 512], F32, tag="rz", name=f"rz{i}")
                    xpt = xpps.tile([128, 512], F32, tag="xp", name=f"xp{i}")
                    rz.append(rzt)
                    xpn.append(xpt)
                    for k in range(4):
                        st = k == 0
                        xk = xt[:, k * W + glo:k * W + ghi]
                        if fw > 0 and rw > 0:
                            # F: start opens the bank (lazy-zeroes it); the
                            # R x-proj piggybacks with start=False and gets
                            # zero-init from the pending-zero region.  The
                            # h-proj's stop closes the bank group.
                            xkF = xt[:, k * W + fl:k * W + fh]
                            xkR = xt[:, k * W + rl:k * W + rh]
                            nc.tensor.matmul(rzt[:, 0, :fw], w[:, k, ro:ro + 128],
                                             xkF, start=st, stop=False)
                            nc.tensor.matmul(rzt[:, 0, fw:gw], w[:, k, ro:ro + 128],
                                             xkR, start=False, stop=False,
                                             skip_group_check=True)
                            nc.tensor.matmul(rzt[:, 1, :fw], w[:, k, zo:zo + 128],
                                             xkF, start=st, stop=False)
                            nc.tensor.matmul(rzt[:, 1, fw:gw], w[:, k, zo:zo + 128],
                                             xkR, start=False, stop=False,
                                             skip_group_check=True)
                        else:
                            sp = k == 3 and rw == 0
                            nc.tensor.matmul(rzt[:, 0, :gw], w[:, k, ro:ro + 128],
                                             xk, start=st, stop=sp)
                            nc.tensor.matmul(rzt[:, 1, :gw], w[:, k, zo:zo + 128],
                                             xk, start=st, stop=sp)
                        nc.tensor.matmul(xpt[:, :gw], w[:, k, no:no + 128],
                                         xk, start=st, stop=k == 3)
                    if rw > 0:
                        ght = ghps.tile([128, 512], F32, tag="gh", name=f"gh{i}")
                        ghn.append(ght)
                        fo = rl - glo  # local-in-group offset of R
                        for k in range(4):
                            hk = h_prev[:, k, goff + rl:goff + rh]
                            nc.tensor.matmul(rzt[:, 0, fo:gw],
                                             w[:, 4 + k, ro:ro + 128], hk,
                                             start=False, stop=k == 3)
                            nc.tensor.matmul(rzt[:, 1, fo:gw],
                                             w[:, 4 + k, zo:zo + 128], hk,
                                             start=False, stop=k == 3)
                            nc.tensor.matmul(ght[:, :rw],
                                             w[:, 4 + k, no:no + 128], hk,
                                             start=k == 0, stop=k == 3)

                # --- elementwise chain for this group, per i-pair so the
                # next step's h-projection k-chunks can start early ---
                rz4 = rz4p.tile([128, 2, 4, 512], DT, tag="rz4", name="rz4")
                ss4 = ssp.tile([128, 4, 512], DT, tag="ss", name="ss4")
                n4 = np_.tile([128, 4, 512], DT, tag="n4", name="n4")
                sc = scr.tile([128, 4, 512], DT, tag="scr", name="sc")

                def upd_pair(p0):
                    ii = slice(p0, p0 + 2)
                    nc.scalar.activation(n4[:, ii, :gw], ss4[:, ii, :gw],
                                         ACT.Tanh)
                    if fw > 0:
                        zF = rz4[:, 1, ii, fl - glo:fh - glo]
                        nF = n4[:, ii, fl - glo:fh - glo]
                        eF = sc[:, ii, fl - glo:fh - glo]
                        nc.vector.tensor_mul(eF, zF, nF)
                        nc.vector.tensor_sub(h_next[:, ii, goff + fl:goff + fh],
                                             nF, eF)
                    if rw > 0:
                        zR = rz4[:, 1, ii, rl - glo:rh - glo]
                        nR = n4[:, ii, rl - glo:rh - glo]
                        dd = sc[:, ii, rl - glo:rh - glo]
                        nc.vector.tensor_sub(dd, h_prev[:, ii, goff + rl:goff + rh],
                                             nR)
                        nc.vector.tensor_mul(dd, zR, dd)
                        nc.vector.tensor_add(h_next[:, ii, goff + rl:goff + rh],
                                             nR, dd)

                for i in range(4):
                    # r = sigmoid(rps + bias_r)
                    nc.scalar.activation(rz4[:, 0, i, :gw], rz[i][:, 0, :gw],
                                         ACT.Sigmoid, bias=bt[:, bb + i:bb + i + 1])
                    # mask strip: force z -> 1 on over-included columns
                    if sw > 0:
                        nc.vector.tensor_add(rz[i][:, 1, sl - glo:sh - glo],
                                             rz[i][:, 1, sl - glo:sh - glo],
                                             mt[:, sl:sh])
                    nc.scalar.activation(rz4[:, 1, i, :gw], rz[i][:, 1, :gw],
                                         ACT.Sigmoid,
                                         bias=bt[:, bb + 4 + i:bb + 5 + i])
                    # n pre-activation: ss = xpn + bih_n + r * (ghn + bhh_n)
                    if fw > 0:
                        t1 = ttp.tile([128, 512], DT, tag="tt", name="t1")
                        nc.vector.tensor_scalar(
                            t1[:, :fw], rz4[:, 0, i, fl - glo:fh - glo],
                            bt[:, bb + 8 + i:bb + 9 + i],
                            bt[:, bb + 12 + i:bb + 13 + i],
                            op0=ALU.mult, op1=ALU.add)
                        nc.vector.tensor_add(
                            ss4[:, i, fl - glo:fh - glo], t1[:, :fw],
                            xpn[i][:, fl - glo:fh - glo])
                    if rw > 0:
                        t2 = ttp.tile([128, 512], DT, tag="tt", name="t2")
                        nc.vector.scalar_tensor_tensor(
                            t2[:, :rw], ghn[i][:, :rw],
                            bt[:, bb + 8 + i:bb + 9 + i],
                            rz4[:, 0, i, rl - glo:rh - glo],
                            op0=ALU.add, op1=ALU.mult)
                        nc.vector.scalar_tensor_tensor(
                            ss4[:, i, rl - glo:rh - glo], t2[:, :rw],
                            bt[:, bb + 12 + i:bb + 13 + i],
                            xpn[i][:, rl - glo:rh - glo],
                            op0=ALU.add, op1=ALU.add)
                    if i == 1:
                        upd_pair(0)
                    elif i == 3:
                        upd_pair(2)
            return h_next

        # Interleave the two directions step-by-step: while one direction's
        # elementwise chain drains, the PE runs the other direction's
        # (independent) matmuls -- kills recurrence-latency bubbles at the
        # narrow early steps.
        hf4 = hb4 = None
        off_f, off_b = 0, 4 * sum(w for w, _ in f_steps)
        for j in range(max(nf, nb)):
            if j < nf:
                xt = emit_x(f_steps, xfp, "xf", j, off_f)
                off_f += 4 * f_steps[j][0]
                hf4 = emit_step(j, f_steps, xt, hf4, wf, mf_d, 0, hfp, "hf",
                                j == nf - 1)
            if j < nb:
                xt = emit_x(b_steps, xbp, "xb", j, off_b)
                off_b += 4 * b_steps[j][0]
                hb4 = emit_step(j, b_steps, xt, hb4, wb, mb_d, 16, hbp, "hb",
                                j == nb - 1)

        # MLP phase A: acc = W1[:, :H].T @ hf -- independent of hb, fills the
        # PE while the last backward step's chain drains.
        acc = accp.tile([128, 4, Bc], DT, tag="mlp", name="acc")
        for g in range(Bc // 512):
            for i in range(4):
                ps = xpps.tile([128, 512], F32, tag="xp", name="mlpA")
                for k in range(4):
                    nc.tensor.matmul(ps[:], w1[:, k, i * 128:(i + 1) * 128],
                                     hf4[:, k, g * 512:(g + 1) * 512],
                                     start=k == 0, stop=k == 3)
                nc.scalar.activation(acc[:, i, g * 512:(g + 1) * 512], ps[:],
                                     ACT.Copy)

        # MLP phases B+C interleaved per column group:
        #   hid = relu(acc + W1[:, H:].T @ hb + b1);  y = W2.T @ hid + b2
        hid = accp.tile([128, 4, Bc], DT, tag="mlp", name="hid")
        for g in range(Bc // 512):
            gs = slice(g * 512, (g + 1) * 512)
            pre = ssp.tile([128, 4, 512], DT, tag="ss", name="pre")
            for i in range(4):
                ps = xpps.tile([128, 512], F32, tag="xp", name="mlpB")
                for k in range(4):
                    nc.tensor.matmul(ps[:], w1[:, 4 + k, i * 128:(i + 1) * 128],
                                     hb4[:, k, gs], start=k == 0, stop=k == 3)
                nc.vector.scalar_tensor_tensor(
                    pre[:, i, :], ps[:], bt[:, 32 + i:33 + i],
                    acc[:, i, gs], op0=ALU.add, op1=ALU.add)
            nc.scalar.activation(hid[:, :, gs], pre[:], ACT.Relu)
            for i in range(4):
                ps = xpps.tile([128, 512], F32, tag="xp", name="mlpC")
                for k in range(4):
                    nc.tensor.matmul(ps[:], w2[:, k, i * 128:(i + 1) * 128],
                                     hid[:, k, gs], start=k == 0, stop=k == 3)
                o32 = obuf.tile([128, 512], F32, tag="o", name="o32")
                nc.scalar.activation(o32[:], ps[:], ACT.Identity,
                                     bias=bt[:, 36 + i:37 + i])
                nc.scalar.dma_start(
                    y_d[i * 128:(i + 1) * 128, gs], o32[:])

    nc.compile()
    return nc


def kernel(padded_window, window_len, Wih_f, Whh_f, bih_f, bhh_f,
           Wih_b, Whh_b, bih_b, bhh_b, W1, b1, W2, b2):
    wl = np.asarray(window_len)
    lf = (wl - 1) // 2 + 1
    lb = wl // 2 + 1
    order = np.argsort(wl, kind="stable")

    # per-core sorted lengths: row k = per-core rank k, column = core
    lf_pc = lf[order].reshape(-1, NCORES)
    lb_pc = lb[order].reshape(-1, NCORES)

    def dir_steps(lens_pc):
        n = int(lens_pc.max())
        steps, cnts = [], []
        for j in range(n):
            need = n - j
            cnt = (lens_pc >= need).sum(axis=0)  # per core
            W = int(cnt.max())
            strip = W - int(cnt.min())
            assert strip <= 256, f"mask strip {strip} exceeds tile"
            steps.append((W, strip))
            cnts.append(cnt)
        return tuple(steps), cnts

    f_steps, f_cnts = dir_steps(lf_pc)
    b_steps, b_cnts = dir_steps(lb_pc)
    sched = (f_steps, b_steps)

    if sched not in _PROGRAM_CACHE:
        _PROGRAM_CACHE[sched] = _build_program(sched)
    nc = _PROGRAM_CACHE[sched]

    f16 = np.float16
    f32 = np.float32
    wf = np.concatenate([Wih_f.T, Whh_f.T], 0).astype(f16)
    wb = np.concatenate([Wih_b.T, Whh_b.T], 0).astype(f16)
    w1 = np.ascontiguousarray(W1.T, dtype=f16)
    w2 = np.ascontiguousarray(W2.T, dtype=f16)

    def chunks(v):  # [512] -> [4, 128]
        return np.asarray(v, f32).reshape(4, 128)

    bias = np.concatenate([
        chunks((bih_f + bhh_f)[:H]), chunks((bih_f + bhh_f)[H:2 * H]),
        chunks(bhh_f[2 * H:]), chunks(bih_f[2 * H:]),
        chunks((bih_b + bhh_b)[:H]), chunks((bih_b + bhh_b)[H:2 * H]),
        chunks(bhh_b[2 * H:]), chunks(bih_b[2 * H:]),
        chunks(b1), chunks(b2),
    ], 0)  # [40, 128]

    pw16 = np.asarray(padded_window).astype(f16)
    pos = np.arange(Bc)
    nf, nb = len(f_steps), len(b_steps)
    in_maps = []
    for c in range(NCORES):
        idx = order[c::NCORES]
        xTc = pw16[idx].transpose(1, 2, 0)  # [15, 512, Bc] (view-ish)
        blocks = []
        for steps, pfn in ((f_steps, lambda j: 8 - nf + j),
                           (b_steps, lambda j: 6 + nb - j)):
            for j, (W, _) in enumerate(steps):
                sl = xTc[pfn(j), :, Bc - W:]  # [512, W]
                blocks.append(sl.reshape(4, 128, W).transpose(1, 0, 2)
                              .reshape(128, 4 * W))
        xpk = np.ascontiguousarray(np.concatenate(blocks, axis=1))
        mzf = np.stack([(BIG * (pos < Bc - cnt[c])).astype(f32)
                        for cnt in f_cnts])
        mzb = np.stack([(BIG * (pos < Bc - cnt[c])).astype(f32)
                        for cnt in b_cnts])
        in_maps.append({
            "xpk": xpk, "wf": wf, "wb": wb, "w1": w1, "w2": w2,
            "bias": bias, "maskzf": mzf, "maskzb": mzb,
        })

    trace = bool(os.environ.get("GRU_TRACE"))
    kw = {}
    if os.environ.get("GRU_TMPDIR"):
        kw["tmpdir"] = os.environ["GRU_TMPDIR"]
    res = run_bass_kernel_spmd(nc, in_maps, core_ids=list(range(NCORES)),
                               trace=trace, **kw)
    global LAST_RESULT
    LAST_RESULT = res
    out = np.empty((B, H), f32)
    for c in range(NCORES):
        out[order[c::NCORES]] = res.results[c]["yT"].T
    return out


# revision 19
# speedup vs baseline: 1.0297x; 1.0202x over previous
"""BiGRU encoder kernel for 8 Trainium2 NeuronCores (fp16, exact ragged schedule).

Strategy:
  - Masked fixed-position reformulation: forward runs positions ascending into
    the center, backward descending into the center, so every sample's
    recurrence ENDS on the final step.  A sample of length l starts updating
    at the step where need == l; before that its hidden state is held at 0
    exactly by forcing z = 1 (+BIG on the z pre-activation).
  - Samples sorted by window_len, dealt round-robin to 8 cores (data
    parallel).  Each core holds ONE sorted batch of 1024 columns
    (features on SBUF partitions, samples on the free dim).  Step j runs on
    the exact suffix W_j = max over cores of #samples with len >= need --
    fp16 matmuls are full rate at any width, so no minimum-width padding.
  - Within a step, columns split into F (samples taking their first step:
    h == 0) and R (already running).  The hidden projection runs ONLY on R;
    F columns take a cheap h'=(1-z)n update that never reads h_prev.
    Cross-core width slack is fixed up by a narrow mask strip on z.
  - The suffix splits into 512-wide groups (PSUM bank limit).  Gate biases
    are folded into scalar_tensor_tensor ops so tanh and the h-update run as
    single wide ops over [128, 4, W].
  - Output is written feature-major (yT) and transposed on the host.
"""

import os
from contextlib import ExitStack

import numpy as np

import concourse.bacc as bacc
import concourse.tile as tile
from concourse import mybir
from concourse.bass_utils import run_bass_kernel_spmd

NCORES = 8
B, T, D, H = 8192, 15, 512, 512
G = 3 * H
Bc = B // NCORES  # 1024 columns per core
BIG = 40.0
F32 = mybir.dt.float32
DT = mybir.dt.float16

ACT = mybir.ActivationFunctionType
ALU = mybir.AluOpType

_PROGRAM_CACHE = {}
LAST_RESULT = None


def _ceil(a, b):
    return -(-a // b)


def _build_program(sched):
    """sched = (f_steps, b_steps); each steps = tuple of (W, strip) per step,
    W monotone nondecreasing, W[-1] == Bc."""
    f_steps, b_steps = sched
    nf, nb = len(f_steps), len(b_steps)

    nc = bacc.Bacc("TRN2", target_bir_lowering=False, debug=False,
                   num_devices=NCORES)

    S4 = 4 * (sum(w for w, _ in f_steps) + sum(w for w, _ in b_steps))
    xpk_d = nc.dram_tensor("xpk", [128, S4], DT, kind="ExternalInput")
    wf_d = nc.dram_tensor("wf", [D + H, G], DT, kind="ExternalInput")
    wb_d = nc.dram_tensor("wb", [D + H, G], DT, kind="ExternalInput")
    w1_d = nc.dram_tensor("w1", [2 * H, H], DT, kind="ExternalInput")
    w2_d = nc.dram_tensor("w2", [H, H], DT, kind="ExternalInput")
    bias_d = nc.dram_tensor("bias", [40, 128], F32, kind="ExternalInput")
    mf_d = nc.dram_tensor("maskzf", [nf, Bc], F32, kind="ExternalInput")
    mb_d = nc.dram_tensor("maskzb", [nb, Bc], F32, kind="ExternalInput")
    y_d = nc.dram_tensor("yT", [H, Bc], DT, kind="ExternalOutput")

    with tile.TileContext(nc) as tc, ExitStack() as ctx:
        const = ctx.enter_context(tc.tile_pool(name="const", bufs=1))
        wpool = ctx.enter_context(tc.tile_pool(name="w", bufs=2))
        xfp = ctx.enter_context(tc.tile_pool(name="xf", bufs=2))
        xbp = ctx.enter_context(tc.tile_pool(name="xb", bufs=2))
        hfp = ctx.enter_context(tc.tile_pool(name="hf", bufs=2))
        hbp = ctx.enter_context(tc.tile_pool(name="hb", bufs=2))
        hfin = ctx.enter_context(tc.tile_pool(name="hfin", bufs=2))
        rz4p = ctx.enter_context(tc.tile_pool(name="rz4", bufs=2))
        ssp = ctx.enter_context(tc.tile_pool(name="ss", bufs=2))
        np_ = ctx.enter_context(tc.tile_pool(name="n4", bufs=2))
        scr = ctx.enter_context(tc.tile_pool(name="scr", bufs=2))
        ttp = ctx.enter_context(tc.tile_pool(name="tt", bufs=4))
        obuf = ctx.enter_context(tc.tile_pool(name="o", bufs=2))
        mpool = ctx.enter_context(tc.tile_pool(name="m", bufs=2))
        accp = ctx.enter_context(tc.tile_pool(name="mlp", bufs=2))
        rzps = ctx.enter_context(tc.tile_pool(name="rz", bufs=2, space="PSUM"))
        xpps = ctx.enter_context(tc.tile_pool(name="xp", bufs=2, space="PSUM"))
        ghps = ctx.enter_context(tc.tile_pool(name="gh", bufs=2, space="PSUM"))

        # Weights as [128, kchunk, gate-cols]; kchunks 0-3 input dims, 4-7
        # hidden dims.  Per-kchunk DMAs so the first matmuls start as soon
        # as chunk 0 lands.
        def load_w(dram, kchunks, cols, name, pool, tag, eng):
            t_ = pool.tile([128, kchunks, cols], DT, tag=tag, name=name)
            src = dram.rearrange("(c k) g -> k c g", k=128)
            for c in range(kchunks):
                eng.dma_start(t_[:, c, :], src[:, c, :])
            return t_

        bt = const.tile([128, 40], F32)
        nc.gpsimd.dma_start(bt[:], bias_d.rearrange("n p -> p n"))
        wf = load_w(wf_d, 8, G, "wf", wpool, "w", nc.scalar)
        wb = load_w(wb_d, 8, G, "wb", wpool, "w", nc.gpsimd)

        def emit_x(steps, pool, tag, j, off):
            # x tile holds the step's suffix packed [4k x W] contiguously per
            # partition -- single fat DMA run on both sides.
            W = steps[j][0]
            xt = pool.tile([128, 4 * Bc], DT, tag=tag, name=f"{tag}{j}")
            nc.sync.dma_start(xt[:, :4 * W], xpk_d[:, off:off + 4 * W])
            return xt

        def emit_step(j, steps, xt, h_prev, w, mask_d, bb, hpool, tag, is_last):
            """One GRU step.  Local cols 0..W-1 map to global Bc-W..Bc-1.
            F = [0, Fw): first-step columns.  R = [Fw, W): running."""
            W, strip = steps[j]
            Wprev = steps[j - 1][0] if j > 0 else 0
            Fw = W - Wprev
            goff = Bc - W  # local -> global

            h_next = (hfin if is_last else hpool).tile(
                [128, 4, Bc], DT, tag="hfin" if is_last else tag, name="h")

            mt = None
            if strip > 0:
                mt = mpool.tile([128, 256], F32, tag="m", name="mt")
                nc.sync.dma_start(
                    mt[:, :strip],
                    mask_d[j, goff:goff + strip].partition_broadcast(128),
                )

            ngroups = _ceil(W, 512)
            for g in range(ngroups):  # left-aligned groups on local coords
                glo, ghi = 512 * g, min(512 * (g + 1), W)
                gw = ghi - glo
                fl, fh = glo, max(min(ghi, Fw), glo)   # F within group
                rl, rh = max(glo, Fw), ghi             # R within group
                fw, rw = fh - fl, rh - rl
                sl, sh = glo, max(min(ghi, strip), glo)  # mask strip in group
                sw = sh - sl

                rz = []
                xpn = []
                ghn = []
                for i in range(4):
                    ro, zo, no = i * 128, H + i * 128, 2 * H + i * 128
                    rzt = rzps.tile([128, 2, 512], F32, tag="rz", name=f"rz{i}")
                    xpt = xpps.tile([128, 512], F32, tag="xp", name=f"xp{i}")
                    rz.append(rzt)
                    xpn.append(xpt)
                    for k in range(4):
                        st = k == 0
                        xk = xt[:, k * W + glo:k * W + ghi]
                        if fw > 0 and rw > 0:
                            # F: start opens the bank (lazy-zeroes it); the
                            # R x-proj piggybacks with start=False and gets
                            # zero-init from the pending-zero region.  The
                            # h-proj's stop closes the bank group.
                            xkF = xt[:, k * W + fl:k * W + fh]
                            xkR = xt[:, k * W + rl:k * W + rh]
                            nc.tensor.matmul(rzt[:, 0, :fw], w[:, k, ro:ro + 128],
                                             xkF, start=st, stop=False)
                            nc.tensor.matmul(rzt[:, 0, fw:gw], w[:, k, ro:ro + 128],
                                             xkR, start=False, stop=False,
                                             skip_group_check=True)
                            nc.tensor.matmul(rzt[:, 1, :fw], w[:, k, zo:zo + 128],
                                             xkF, start=st, stop=False)
                            nc.tensor.matmul(rzt[:, 1, fw:gw], w[:, k, zo:zo + 128],
                                             xkR, start=False, stop=False,
                                             skip_group_check=True)
                        else:
                            sp = k == 3 and rw == 0
                            nc.tensor.matmul(rzt[:, 0, :gw], w[:, k, ro:ro + 128],
                                             xk, start=st, stop=sp)
                            nc.tensor.matmul(rzt[:, 1, :gw], w[:, k, zo:zo + 128],
                                             xk, start=st, stop=sp)
                        nc.tensor.matmul(xpt[:, :gw], w[:, k, no:no + 128],
                                         xk, start=st, stop=k == 3)
                    if rw > 0:
                        ght = ghps.tile([128, 512], F32, tag="gh", name=f"gh{i}")
                        ghn.append(ght)
                        fo = rl - glo  # local-in-group offset of R
                        for k in range(4):
                            hk = h_prev[:, k, goff + rl:goff + rh]
                            nc.tensor.matmul(rzt[:, 0, fo:gw],
                                             w[:, 4 + k, ro:ro + 128], hk,
                                             start=False, stop=k == 3)
                            nc.tensor.matmul(rzt[:, 1, fo:gw],
                                             w[:, 4 + k, zo:zo + 128], hk,
                                             start=False, stop=k == 3)
                            nc.tensor.matmul(ght[:, :rw],
                                             w[:, 4 + k, no:no + 128], hk,
                                             start=k == 0, stop=k == 3)

                # --- elementwise chain for this group, per i-pair so the
                # next step's h-projection k-chunks can start early ---
                rz4 = rz4p.tile([128, 2, 4, 512], DT, tag="rz4", name="rz4")
                ss4 = ssp.tile([128, 4, 512], DT, tag="ss", name="ss4")
                n4 = np_.tile([128, 4, 512], DT, tag="n4", name="n4")
                sc = scr.tile([128, 4, 512], DT, tag="scr", name="sc")

                def upd_pair(p0):
                    ii = slice(p0, p0 + 2)
                    nc.scalar.activation(n4[:, ii, :gw], ss4[:, ii, :gw],
                                         ACT.Tanh)
                    if fw > 0:
                        zF = rz4[:, 1, ii, fl - glo:fh - glo]
                        nF = n4[:, ii, fl - glo:fh - glo]
                        eF = sc[:, ii, fl - glo:fh - glo]
                        nc.vector.tensor_mul(eF, zF, nF)
                        nc.vector.tensor_sub(h_next[:, ii, goff + fl:goff + fh],
                                             nF, eF)
                    if rw > 0:
                        zR = rz4[:, 1, ii, rl - glo:rh - glo]
                        nR = n4[:, ii, rl - glo:rh - glo]
                        dd = sc[:, ii, rl - glo:rh - glo]
                        nc.vector.tensor_sub(dd, h_prev[:, ii, goff + rl:goff + rh],
                                             nR)
                        nc.vector.tensor_mul(dd, zR, dd)
                        nc.vector.tensor_add(h_next[:, ii, goff + rl:goff + rh],
                                             nR, dd)

                for i in range(4):
                    # r = sigmoid(rps + bias_r)
                    nc.scalar.activation(rz4[:, 0, i, :gw], rz[i][:, 0, :gw],
                                         ACT.Sigmoid, bias=bt[:, bb + i:bb + i + 1])
                    # mask strip: force z -> 1 on over-included columns
                    if sw > 0:
                        nc.vector.tensor_add(rz[i][:, 1, sl - glo:sh - glo],
                                             rz[i][:, 1, sl - glo:sh - glo],
                                             mt[:, sl:sh])
                    nc.scalar.activation(rz4[:, 1, i, :gw], rz[i][:, 1, :gw],
                                         ACT.Sigmoid,
                                         bias=bt[:, bb + 4 + i:bb + 5 + i])
                    # n pre-activation: ss = xpn + bih_n + r * (ghn + bhh_n)
                    if fw > 0:
                        t1 = ttp.tile([128, 512], DT, tag="tt", name="t1")
                        nc.vector.tensor_scalar(
                            t1[:, :fw], rz4[:, 0, i, fl - glo:fh - glo],
                            bt[:, bb + 8 + i:bb + 9 + i],
                            bt[:, bb + 12 + i:bb + 13 + i],
                            op0=ALU.mult, op1=ALU.add)
                        nc.vector.tensor_add(
                            ss4[:, i, fl - glo:fh - glo], t1[:, :fw],
                            xpn[i][:, fl - glo:fh - glo])
                    if rw > 0:
                        t2 = ttp.tile([128, 512], DT, tag="tt", name="t2")
                        nc.vector.scalar_tensor_tensor(
                            t2[:, :rw], ghn[i][:, :rw],
                            bt[:, bb + 8 + i:bb + 9 + i],
                            rz4[:, 0, i, rl - glo:rh - glo],
                            op0=ALU.add, op1=ALU.mult)
                        nc.vector.scalar_tensor_tensor(
                            ss4[:, i, rl - glo:rh - glo], t2[:, :rw],
                            bt[:, bb + 12 + i:bb + 13 + i],
                            xpn[i][:, rl - glo:rh - glo],
                            op0=ALU.add, op1=ALU.add)
                    if i == 1:
                        upd_pair(0)
                    elif i == 3:
                        upd_pair(2)
            return h_next

        # Interleave the two directions step-by-step: while one direction's
        # elementwise chain drains, the PE runs the other direction's
        # (independent) matmuls -- kills recurrence-latency bubbles at the
        # narrow early steps.
        hf4 = hb4 = None
        off_f, off_b = 0, 4 * sum(w for w, _ in f_steps)
        for j in range(max(nf, nb)):
            if j < nf:
                xt = emit_x(f_steps, xfp, "xf", j, off_f)
                off_f += 4 * f_steps[j][0]
                hf4 = emit_step(j, f_steps, xt, hf4, wf, mf_d, 0, hfp, "hf",
                                j == nf - 1)
            if j < nb:
                xt = emit_x(b_steps, xbp, "xb", j, off_b)
                off_b += 4 * b_steps[j][0]
                hb4 = emit_step(j, b_steps, xt, hb4, wb, mb_d, 16, hbp, "hb",
                                j == nb - 1)

        # w1/w2 load mid-kernel: keeps the HBM-bound warmup window clear
        w1 = load_w(w1_d, 8, H, "w1", const, "w1", nc.gpsimd)
        w2 = load_w(w2_d, 4, H, "w2", const, "w2", nc.gpsimd)

        # MLP phase A: acc = W1[:, :H].T @ hf -- independent of hb, fills the
        # PE while the last backward step's chain drains.  Uses the ghps PSUM
        # pool (freed early in the GRU chain) to avoid stalling on the last
        # backward step's xpn consumers.
        acc = accp.tile([128, 4, Bc], DT, tag="mlp", name="acc")
        for g in range(Bc // 512):
            for i in range(4):
                ps = ghps.tile([128, 512], F32, tag="gh", name="mlpA")
                for k in range(4):
                    nc.tensor.matmul(ps[:], w1[:, k, i * 128:(i + 1) * 128],
                                     hf4[:, k, g * 512:(g + 1) * 512],
                                     start=k == 0, stop=k == 3)
                nc.scalar.activation(acc[:, i, g * 512:(g + 1) * 512], ps[:],
                                     ACT.Copy)

        # MLP phase B: hid = relu(acc + W1[:, H:].T @ hb + b1)
        hid = accp.tile([128, 4, Bc], DT, tag="mlp", name="hid")
        for g in range(Bc // 512):
            gs = slice(g * 512, (g + 1) * 512)
            pre = ssp.tile([128, 4, 512], DT, tag="ss", name="pre")
            for i in range(4):
                ps = xpps.tile([128, 512], F32, tag="xp", name="mlpB")
                for k in range(4):
                    nc.tensor.matmul(ps[:], w1[:, 4 + k, i * 128:(i + 1) * 128],
                                     hb4[:, k, gs], start=k == 0, stop=k == 3)
                nc.vector.scalar_tensor_tensor(
                    pre[:, i, :], ps[:], bt[:, 32 + i:33 + i],
                    acc[:, i, gs], op0=ALU.add, op1=ALU.add)
            nc.scalar.activation(hid[:, :, gs], pre[:], ACT.Relu)
        # MLP phase C: y = W2.T @ hid + b2, written feature-major fp16
        for g in range(Bc // 512):
            gs = slice(g * 512, (g + 1) * 512)
            for i in range(4):
                ps = ghps.tile([128, 512], F32, tag="gh", name="mlpC")
                for k in range(4):
                    nc.tensor.matmul(ps[:], w2[:, k, i * 128:(i + 1) * 128],
                                     hid[:, k, gs], start=k == 0, stop=k == 3)
                o32 = obuf.tile([128, 512], DT, tag="o", name="o32")
                nc.scalar.activation(o32[:], ps[:], ACT.Identity,
                                     bias=bt[:, 36 + i:37 + i])
                nc.scalar.dma_start(
                    y_d[i * 128:(i + 1) * 128, gs], o32[:])

    nc.compile()
    return nc


def kernel(padded_window, window_len, Wih_f, Whh_f, bih_f, bhh_f,
           Wih_b, Whh_b, bih_b, bhh_b, W1, b1, W2, b2):
    wl = np.asarray(window_len)
    lf = (wl - 1) // 2 + 1
    lb = wl // 2 + 1
    order = np.argsort(wl, kind="stable")

    # per-core sorted lengths: row k = per-core rank k, column = core
    lf_pc = lf[order].reshape(-1, NCORES)
    lb_pc = lb[order].reshape(-1, NCORES)

    def dir_steps(lens_pc):
        n = int(lens_pc.max())
        steps, cnts = [], []
        for j in range(n):
            need = n - j
            cnt = (lens_pc >= need).sum(axis=0)  # per core
            W = int(cnt.max())
            strip = W - int(cnt.min())
            assert strip <= 256, f"mask strip {strip} exceeds tile"
            steps.append((W, strip))
            cnts.append(cnt)
        return tuple(steps), cnts

    f_steps, f_cnts = dir_steps(lf_pc)
    b_steps, b_cnts = dir_steps(lb_pc)
    sched = (f_steps, b_steps)

    if sched not in _PROGRAM_CACHE:
        _PROGRAM_CACHE[sched] = _build_program(sched)
    nc = _PROGRAM_CACHE[sched]

    f16 = np.float16
    f32 = np.float32
    wf = np.concatenate([Wih_f.T, Whh_f.T], 0).astype(f16)
    wb = np.concatenate([Wih_b.T, Whh_b.T], 0).astype(f16)
    w1 = np.ascontiguousarray(W1.T, dtype=f16)
    w2 = np.ascontiguousarray(W2.T, dtype=f16)

    def chunks(v):  # [512] -> [4, 128]
        return np.asarray(v, f32).reshape(4, 128)

    bias = np.concatenate([
        chunks((bih_f + bhh_f)[:H]), chunks((bih_f + bhh_f)[H:2 * H]),
        chunks(bhh_f[2 * H:]), chunks(bih_f[2 * H:]),
        chunks((bih_b + bhh_b)[:H]), chunks((bih_b + bhh_b)[H:2 * H]),
        chunks(bhh_b[2 * H:]), chunks(bih_b[2 * H:]),
        chunks(b1), chunks(b2),
    ], 0)  # [40, 128]

    pw16 = np.asarray(padded_window).astype(f16)
    pos = np.arange(Bc)
    nf, nb = len(f_steps), len(b_steps)
    in_maps = []
    for c in range(NCORES):
        idx = order[c::NCORES]
        xTc = pw16[idx].transpose(1, 2, 0)  # [15, 512, Bc] (view-ish)
        blocks = []
        for steps, pfn in ((f_steps, lambda j: 8 - nf + j),
                           (b_steps, lambda j: 6 + nb - j)):
            for j, (W, _) in enumerate(steps):
                sl = xTc[pfn(j), :, Bc - W:]  # [512, W]
                blocks.append(sl.reshape(4, 128, W).transpose(1, 0, 2)
                              .reshape(128, 4 * W))
        xpk = np.ascontiguousarray(np.concatenate(blocks, axis=1))
        mzf = np.stack([(BIG * (pos < Bc - cnt[c])).astype(f32)
                        for cnt in f_cnts])
        mzb = np.stack([(BIG * (pos < Bc - cnt[c])).astype(f32)
                        for cnt in b_cnts])
        in_maps.append({
            "xpk": xpk, "wf": wf, "wb": wb, "w1": w1, "w2": w2,
            "bias": bias, "maskzf": mzf, "maskzb": mzb,
        })

    trace = bool(os.environ.get("GRU_TRACE"))
    kw = {}
    if os.environ.get("GRU_TMPDIR"):
        kw["tmpdir"] = os.environ["GRU_TMPDIR"]
    res = run_bass_kernel_spmd(nc, in_maps, core_ids=list(range(NCORES)),
                               trace=trace, **kw)
    global LAST_RESULT
    LAST_RESULT = res
    out = np.empty((B, H), f32)
    for c in range(NCORES):
        out[order[c::NCORES]] = res.results[c]["yT"].T
    return out


# revision 23
# speedup vs baseline: 1.3106x; 1.2728x over previous
"""BiGRU encoder kernel for 8 Trainium2 NeuronCores (fp16, exact ragged schedule).

Strategy:
  - Masked fixed-position reformulation: forward runs positions ascending into
    the center, backward descending into the center, so every sample's
    recurrence ENDS on the final step.  A sample of length l starts updating
    at the step where need == l; before that its hidden state is held at 0
    exactly by forcing z = 1 (+BIG on the z pre-activation).
  - Samples sorted by window_len, dealt round-robin to 8 cores (data
    parallel).  Each core holds ONE sorted batch of 1024 columns
    (features on SBUF partitions, samples on the free dim).  Step j runs on
    the exact suffix W_j = max over cores of #samples with len >= need --
    fp16 matmuls are full rate at any width, so no minimum-width padding.
  - Within a step, columns split into F (samples taking their first step:
    h == 0) and R (already running).  The hidden projection runs ONLY on R;
    F columns take a cheap h'=(1-z)n update that never reads h_prev.
    Cross-core width slack is fixed up by a narrow mask strip on z.
  - The suffix splits into 512-wide groups (PSUM bank limit).  Gate biases
    are folded into scalar_tensor_tensor ops so tanh and the h-update run as
    single wide ops over [128, 4, W].
  - Output is written feature-major (yT) and transposed on the host.
"""

import os
from contextlib import ExitStack

import numpy as np

import concourse.bacc as bacc
import concourse.tile as tile
from concourse import mybir
from concourse.bass_utils import run_bass_kernel_spmd

NCORES = 8
B, T, D, H = 8192, 15, 512, 512
G = 3 * H
Bc = B // NCORES  # 1024 columns per core
BIG = 40.0
F32 = mybir.dt.float32
DT = mybir.dt.float16
DT8 = mybir.dt.float8e4
DR = mybir.MatmulPerfMode.DoubleRow
WSCALE = 32.0

ACT = mybir.ActivationFunctionType
ALU = mybir.AluOpType

_PROGRAM_CACHE = {}
LAST_RESULT = None


def _ceil(a, b):
    return -(-a // b)


def _build_program(sched):
    """sched = (f_steps, b_steps); each steps = tuple of (W, strip) per step,
    W monotone nondecreasing, W[-1] == Bc."""
    f_steps, b_steps = sched
    nf, nb = len(f_steps), len(b_steps)

    nc = bacc.Bacc("TRN2", target_bir_lowering=False, debug=False,
                   num_devices=NCORES)

    S4 = 4 * (sum(w for w, _ in f_steps) + sum(w for w, _ in b_steps))
    xpk_d = nc.dram_tensor("xpk", [128, S4], DT, kind="ExternalInput")
    wf_d = nc.dram_tensor("wf", [D, G], DT, kind="ExternalInput")
    wb_d = nc.dram_tensor("wb", [D, G], DT, kind="ExternalInput")
    wf8_d = nc.dram_tensor("wf8", [H, G], DT8, kind="ExternalInput")
    wb8_d = nc.dram_tensor("wb8", [H, G], DT8, kind="ExternalInput")
    w1_d = nc.dram_tensor("w1", [2 * H, H], DT, kind="ExternalInput")
    w2_d = nc.dram_tensor("w2", [H, H], DT, kind="ExternalInput")
    bias_d = nc.dram_tensor("bias", [40, 128], F32, kind="ExternalInput")
    mf_d = nc.dram_tensor("maskzf", [nf, Bc], F32, kind="ExternalInput")
    mb_d = nc.dram_tensor("maskzb", [nb, Bc], F32, kind="ExternalInput")
    y_d = nc.dram_tensor("yT", [H, Bc], DT, kind="ExternalOutput")

    with tile.TileContext(nc) as tc, ExitStack() as ctx:
        const = ctx.enter_context(tc.tile_pool(name="const", bufs=1))
        wpool = ctx.enter_context(tc.tile_pool(name="w", bufs=2))
        w8pool = ctx.enter_context(tc.tile_pool(name="w8", bufs=2))
        h8fp = ctx.enter_context(tc.tile_pool(name="h8f", bufs=2))
        h8bp = ctx.enter_context(tc.tile_pool(name="h8b", bufs=2))
        xfp = ctx.enter_context(tc.tile_pool(name="xf", bufs=2))
        xbp = ctx.enter_context(tc.tile_pool(name="xb", bufs=2))
        hfp = ctx.enter_context(tc.tile_pool(name="hf", bufs=2))
        hbp = ctx.enter_context(tc.tile_pool(name="hb", bufs=2))
        hfin = ctx.enter_context(tc.tile_pool(name="hfin", bufs=2))
        rz4p = ctx.enter_context(tc.tile_pool(name="rz4", bufs=2))
        ssp = ctx.enter_context(tc.tile_pool(name="ss", bufs=2))
        np_ = ctx.enter_context(tc.tile_pool(name="n4", bufs=2))
        scr = ctx.enter_context(tc.tile_pool(name="scr", bufs=2))
        ttp = ctx.enter_context(tc.tile_pool(name="tt", bufs=3))
        obuf = ctx.enter_context(tc.tile_pool(name="o", bufs=2))
        mpool = ctx.enter_context(tc.tile_pool(name="m", bufs=2))
        accp = ctx.enter_context(tc.tile_pool(name="mlp", bufs=2))
        rzps = ctx.enter_context(tc.tile_pool(name="rz", bufs=2, space="PSUM"))
        xpps = ctx.enter_context(tc.tile_pool(name="xp", bufs=2, space="PSUM"))
        ghps = ctx.enter_context(tc.tile_pool(name="gh", bufs=2, space="PSUM"))

        # Weights as [128, kchunk, gate-cols]; kchunks 0-3 input dims, 4-7
        # hidden dims.  Per-kchunk DMAs so the first matmuls start as soon
        # as chunk 0 lands.
        def load_w(dram, kchunks, cols, name, pool, tag, eng):
            t_ = pool.tile([128, kchunks, cols], DT, tag=tag, name=name)
            src = dram.rearrange("(c k) g -> k c g", k=128)
            for c in range(kchunks):
                eng.dma_start(t_[:, c, :], src[:, c, :])
            return t_

        bt = const.tile([128, 40], F32)
        nc.gpsimd.dma_start(bt[:], bias_d.rearrange("n p -> p n"))
        wf = load_w(wf_d, 4, G, "wf", wpool, "w", nc.scalar)
        wb = load_w(wb_d, 4, G, "wb", wpool, "w", nc.gpsimd)
        wf8 = w8pool.tile([128, 4, G], DT8, tag="w8", name="wf8")
        wb8 = w8pool.tile([128, 4, G], DT8, tag="w8", name="wb8")
        for c in range(4):
            nc.scalar.dma_start(
                wf8[:, c, :], wf8_d.rearrange("(c k) g -> k c g", k=128)[:, c, :])
        for c in range(4):
            nc.gpsimd.dma_start(
                wb8[:, c, :], wb8_d.rearrange("(c k) g -> k c g", k=128)[:, c, :])

        def emit_x(steps, pool, tag, j, off):
            # x tile holds the step's suffix packed [4k x W] contiguously per
            # partition -- single fat DMA run on both sides.
            W = steps[j][0]
            xt = pool.tile([128, 4 * Bc], DT, tag=tag, name=f"{tag}{j}")
            nc.sync.dma_start(xt[:, :4 * W], xpk_d[:, off:off + 4 * W])
            return xt

        def emit_step(j, steps, xt, hh_prev, w, w8, mask_d, bb, hpool, h8pool,
                      tag, is_last):
            """One GRU step.  Local cols 0..W-1 map to global Bc-W..Bc-1.
            F = [0, Fw): first-step columns.  R = [Fw, W): running."""
            W, strip = steps[j]
            Wprev = steps[j - 1][0] if j > 0 else 0
            Fw = W - Wprev
            goff = Bc - W  # local -> global
            h_prev, h8_prev = hh_prev if hh_prev else (None, None)

            h_next = (hfin if is_last else hpool).tile(
                [128, 4, Bc], DT, tag="hfin" if is_last else tag, name="h")
            h8_next = None
            if not is_last:
                h8_next = h8pool.tile([128, 4, Bc], DT8, tag=tag + "8",
                                      name="h8")

            mt = None
            if strip > 0:
                mt = mpool.tile([128, 64], F32, tag="m", name="mt")
                nc.sync.dma_start(
                    mt[:, :strip],
                    mask_d[j, goff:goff + strip].partition_broadcast(128),
                )

            ngroups = _ceil(W, 512)
            for g in range(ngroups):  # left-aligned groups on local coords
                glo, ghi = 512 * g, min(512 * (g + 1), W)
                gw = ghi - glo
                fl, fh = glo, max(min(ghi, Fw), glo)   # F within group
                rl, rh = max(glo, Fw), ghi             # R within group
                fw, rw = fh - fl, rh - rl
                sl, sh = glo, max(min(ghi, strip), glo)  # mask strip in group
                sw = sh - sl

                rz = []
                xpn = []
                ghn = []
                for i in range(4):
                    ro, zo, no = i * 128, H + i * 128, 2 * H + i * 128
                    rzt = rzps.tile([128, 2, 512], F32, tag="rz", name=f"rz{i}")
                    xpt = xpps.tile([128, 512], F32, tag="xp", name=f"xp{i}")
                    rz.append(rzt)
                    xpn.append(xpt)
                    for k in range(4):
                        st = k == 0
                        xk = xt[:, k * W + glo:k * W + ghi]
                        if fw > 0 and rw > 0:
                            # F: start opens the bank (lazy-zeroes it); the
                            # R x-proj piggybacks with start=False and gets
                            # zero-init from the pending-zero region.  The
                            # h-proj's stop closes the bank group.
                            xkF = xt[:, k * W + fl:k * W + fh]
                            xkR = xt[:, k * W + rl:k * W + rh]
                            nc.tensor.matmul(rzt[:, 0, :fw], w[:, k, ro:ro + 128],
                                             xkF, start=st, stop=False)
                            nc.tensor.matmul(rzt[:, 0, fw:gw], w[:, k, ro:ro + 128],
                                             xkR, start=False, stop=False,
                                             skip_group_check=True)
                            nc.tensor.matmul(rzt[:, 1, :fw], w[:, k, zo:zo + 128],
                                             xkF, start=st, stop=False)
                            nc.tensor.matmul(rzt[:, 1, fw:gw], w[:, k, zo:zo + 128],
                                             xkR, start=False, stop=False,
                                             skip_group_check=True)
                        else:
                            sp = k == 3 and rw == 0
                            nc.tensor.matmul(rzt[:, 0, :gw], w[:, k, ro:ro + 128],
                                             xk, start=st, stop=sp)
                            nc.tensor.matmul(rzt[:, 1, :gw], w[:, k, zo:zo + 128],
                                             xk, start=st, stop=sp)
                        nc.tensor.matmul(xpt[:, :gw], w[:, k, no:no + 128],
                                         xk, start=st, stop=k == 3)
                    if rw > 0:
                        ght = ghps.tile([128, 512], F32, tag="gh", name=f"gh{i}")
                        ghn.append(ght)
                        fo = rl - glo  # local-in-group offset of R
                        for p in range(2):  # fp8 DoubleRow: 2 k-chunks/pass
                            hk = h8_prev[:, 2 * p:2 * p + 2,
                                         goff + rl:goff + rh]
                            nc.tensor.matmul(rzt[:, 0, fo:gw],
                                             w8[:, 2 * p:2 * p + 2, ro:ro + 128],
                                             hk, start=False, stop=p == 1,
                                             perf_mode=DR)
                            nc.tensor.matmul(rzt[:, 1, fo:gw],
                                             w8[:, 2 * p:2 * p + 2, zo:zo + 128],
                                             hk, start=False, stop=p == 1,
                                             perf_mode=DR)
                            nc.tensor.matmul(ght[:, :rw],
                                             w8[:, 2 * p:2 * p + 2, no:no + 128],
                                             hk, start=p == 0, stop=p == 1,
                                             perf_mode=DR)

                # --- elementwise chain for this group, per i-pair so the
                # next step's h-projection k-chunks can start early ---
                rz4 = rz4p.tile([128, 2, 4, 512], DT, tag="rz4", name="rz4")
                ss4 = ssp.tile([128, 4, 512], DT, tag="ss", name="ss4")
                n4 = np_.tile([128, 4, 512], DT, tag="n4", name="n4")
                sc = scr.tile([128, 4, 512], DT, tag="scr", name="sc")

                def upd_pair(p0):
                    ii = slice(p0, p0 + 2)
                    nc.scalar.activation(n4[:, ii, :gw], ss4[:, ii, :gw],
                                         ACT.Tanh, scale=1.0 / WSCALE)
                    if fw > 0:
                        zF = rz4[:, 1, ii, fl - glo:fh - glo]
                        nF = n4[:, ii, fl - glo:fh - glo]
                        eF = sc[:, ii, fl - glo:fh - glo]
                        nc.vector.tensor_mul(eF, zF, nF)
                        nc.vector.tensor_sub(h_next[:, ii, goff + fl:goff + fh],
                                             nF, eF)
                    if rw > 0:
                        zR = rz4[:, 1, ii, rl - glo:rh - glo]
                        nR = n4[:, ii, rl - glo:rh - glo]
                        dd = sc[:, ii, rl - glo:rh - glo]
                        nc.vector.tensor_sub(dd, h_prev[:, ii, goff + rl:goff + rh],
                                             nR)
                        nc.vector.tensor_mul(dd, zR, dd)
                        nc.vector.tensor_add(h_next[:, ii, goff + rl:goff + rh],
                                             nR, dd)
                    if h8_next is not None:
                        nc.vector.tensor_copy(
                            h8_next[:, ii, goff + glo:goff + ghi],
                            h_next[:, ii, goff + glo:goff + ghi])

                for i in range(4):
                    # r = sigmoid(rps + bias_r)
                    nc.scalar.activation(rz4[:, 0, i, :gw], rz[i][:, 0, :gw],
                                         ACT.Sigmoid, bias=bt[:, bb + i:bb + i + 1],
                                         scale=1.0 / WSCALE)
                    # mask strip: force z -> 1 on over-included columns
                    if sw > 0:
                        nc.vector.tensor_add(rz[i][:, 1, sl - glo:sh - glo],
                                             rz[i][:, 1, sl - glo:sh - glo],
                                             mt[:, sl:sh])
                    nc.scalar.activation(rz4[:, 1, i, :gw], rz[i][:, 1, :gw],
                                         ACT.Sigmoid,
                                         bias=bt[:, bb + 4 + i:bb + 5 + i],
                                         scale=1.0 / WSCALE)
                    # n pre-activation: ss = xpn + bih_n + r * (ghn + bhh_n)
                    if fw > 0:
                        t1 = ttp.tile([128, 512], DT, tag="tt", name="t1")
                        nc.vector.tensor_scalar(
                            t1[:, :fw], rz4[:, 0, i, fl - glo:fh - glo],
                            bt[:, bb + 8 + i:bb + 9 + i],
                            bt[:, bb + 12 + i:bb + 13 + i],
                            op0=ALU.mult, op1=ALU.add)
                        nc.vector.tensor_add(
                            ss4[:, i, fl - glo:fh - glo], t1[:, :fw],
                            xpn[i][:, fl - glo:fh - glo])
                    if rw > 0:
                        t2 = ttp.tile([128, 512], DT, tag="tt", name="t2")
                        nc.vector.scalar_tensor_tensor(
                            t2[:, :rw], ghn[i][:, :rw],
                            bt[:, bb + 8 + i:bb + 9 + i],
                            rz4[:, 0, i, rl - glo:rh - glo],
                            op0=ALU.add, op1=ALU.mult)
                        nc.vector.scalar_tensor_tensor(
                            ss4[:, i, rl - glo:rh - glo], t2[:, :rw],
                            bt[:, bb + 12 + i:bb + 13 + i],
                            xpn[i][:, rl - glo:rh - glo],
                            op0=ALU.add, op1=ALU.add)
                    if i == 1:
                        upd_pair(0)
                    elif i == 3:
                        upd_pair(2)
            return h_next, h8_next

        # Interleave the two directions step-by-step: while one direction's
        # elementwise chain drains, the PE runs the other direction's
        # (independent) matmuls -- kills recurrence-latency bubbles at the
        # narrow early steps.
        hhf = hhb = None
        off_f, off_b = 0, 4 * sum(w for w, _ in f_steps)
        for j in range(max(nf, nb)):
            if j < nf:
                xt = emit_x(f_steps, xfp, "xf", j, off_f)
                off_f += 4 * f_steps[j][0]
                hhf = emit_step(j, f_steps, xt, hhf, wf, wf8, mf_d, 0,
                                hfp, h8fp, "hf", j == nf - 1)
            if j < nb:
                xt = emit_x(b_steps, xbp, "xb", j, off_b)
                off_b += 4 * b_steps[j][0]
                hhb = emit_step(j, b_steps, xt, hhb, wb, wb8, mb_d, 16,
                                hbp, h8bp, "hb", j == nb - 1)
        hf4, hb4 = hhf[0], hhb[0]

        # w1/w2 load mid-kernel: keeps the HBM-bound warmup window clear
        w1 = load_w(w1_d, 8, H, "w1", const, "w1", nc.gpsimd)
        w2 = load_w(w2_d, 4, H, "w2", const, "w2", nc.gpsimd)

        # MLP phase A: acc = W1[:, :H].T @ hf -- independent of hb, fills the
        # PE while the last backward step's chain drains.  Uses the ghps PSUM
        # pool (freed early in the GRU chain) to avoid stalling on the last
        # backward step's xpn consumers.
        acc = accp.tile([128, 4, Bc], DT, tag="mlp", name="acc")
        for g in range(Bc // 512):
            for i in range(4):
                ps = ghps.tile([128, 512], F32, tag="gh", name="mlpA")
                for k in range(4):
                    nc.tensor.matmul(ps[:], w1[:, k, i * 128:(i + 1) * 128],
                                     hf4[:, k, g * 512:(g + 1) * 512],
                                     start=k == 0, stop=k == 3)
                nc.scalar.activation(acc[:, i, g * 512:(g + 1) * 512], ps[:],
                                     ACT.Copy)

        # MLP phase B: hid = relu(acc + W1[:, H:].T @ hb + b1)
        hid = accp.tile([128, 4, Bc], DT, tag="mlp", name="hid")
        for g in range(Bc // 512):
            gs = slice(g * 512, (g + 1) * 512)
            pre = ssp.tile([128, 4, 512], DT, tag="ss", name="pre")
            for i in range(4):
                ps = xpps.tile([128, 512], F32, tag="xp", name="mlpB")
                for k in range(4):
                    nc.tensor.matmul(ps[:], w1[:, 4 + k, i * 128:(i + 1) * 128],
                                     hb4[:, k, gs], start=k == 0, stop=k == 3)
                nc.vector.scalar_tensor_tensor(
                    pre[:, i, :], ps[:], bt[:, 32 + i:33 + i],
                    acc[:, i, gs], op0=ALU.add, op1=ALU.add)
            nc.scalar.activation(hid[:, :, gs], pre[:], ACT.Relu)
        # MLP phase C: y = W2.T @ hid + b2, written feature-major fp16
        for g in range(Bc // 512):
            gs = slice(g * 512, (g + 1) * 512)
            for i in range(4):
                ps = ghps.tile([128, 512], F32, tag="gh", name="mlpC")
                for k in range(4):
                    nc.tensor.matmul(ps[:], w2[:, k, i * 128:(i + 1) * 128],
                                     hid[:, k, gs], start=k == 0, stop=k == 3)
                o32 = obuf.tile([128, 512], DT, tag="o", name="o32")
                nc.scalar.activation(o32[:], ps[:], ACT.Identity,
                                     bias=bt[:, 36 + i:37 + i])
                nc.scalar.dma_start(
                    y_d[i * 128:(i + 1) * 128, gs], o32[:])

    nc.compile()
    return nc


def kernel(padded_window, window_len, Wih_f, Whh_f, bih_f, bhh_f,
           Wih_b, Whh_b, bih_b, bhh_b, W1, b1, W2, b2):
    wl = np.asarray(window_len)
    lf = (wl - 1) // 2 + 1
    lb = wl // 2 + 1
    order = np.argsort(wl, kind="stable")

    # per-core sorted lengths: row k = per-core rank k, column = core
    lf_pc = lf[order].reshape(-1, NCORES)
    lb_pc = lb[order].reshape(-1, NCORES)

    def dir_steps(lens_pc):
        n = int(lens_pc.max())
        steps, cnts = [], []
        for j in range(n):
            need = n - j
            cnt = (lens_pc >= need).sum(axis=0)  # per core
            W = int(cnt.max())
            strip = W - int(cnt.min())
            assert strip <= 64, f"mask strip {strip} exceeds tile"
            steps.append((W, strip))
            cnts.append(cnt)
        return tuple(steps), cnts

    f_steps, f_cnts = dir_steps(lf_pc)
    b_steps, b_cnts = dir_steps(lb_pc)
    sched = (f_steps, b_steps)

    if sched not in _PROGRAM_CACHE:
        _PROGRAM_CACHE[sched] = _build_program(sched)
    nc = _PROGRAM_CACHE[sched]

    import ml_dtypes
    f16 = np.float16
    f32 = np.float32
    f8 = ml_dtypes.float8_e4m3
    wf = np.ascontiguousarray(Wih_f.T * WSCALE, dtype=f16)
    wb = np.ascontiguousarray(Wih_b.T * WSCALE, dtype=f16)
    wf8 = np.ascontiguousarray((Whh_f.T * WSCALE).astype(f8))
    wb8 = np.ascontiguousarray((Whh_b.T * WSCALE).astype(f8))
    w1 = np.ascontiguousarray(W1.T, dtype=f16)
    w2 = np.ascontiguousarray(W2.T, dtype=f16)

    def chunks(v):  # [512] -> [4, 128]
        return np.asarray(v, f32).reshape(4, 128)

    bias = np.concatenate([
        chunks((bih_f + bhh_f)[:H]), chunks((bih_f + bhh_f)[H:2 * H]),
        chunks(WSCALE * bhh_f[2 * H:]), chunks(WSCALE * bih_f[2 * H:]),
        chunks((bih_b + bhh_b)[:H]), chunks((bih_b + bhh_b)[H:2 * H]),
        chunks(WSCALE * bhh_b[2 * H:]), chunks(WSCALE * bih_b[2 * H:]),
        chunks(b1), chunks(b2),
    ], 0)  # [40, 128]

    pw16 = np.asarray(padded_window).astype(f16)
    pos = np.arange(Bc)
    nf, nb = len(f_steps), len(b_steps)
    in_maps = []
    for c in range(NCORES):
        idx = order[c::NCORES]
        xTc = pw16[idx].transpose(1, 2, 0)  # [15, 512, Bc] (view-ish)
        blocks = []
        for steps, pfn in ((f_steps, lambda j: 8 - nf + j),
                           (b_steps, lambda j: 6 + nb - j)):
            for j, (W, _) in enumerate(steps):
                sl = xTc[pfn(j), :, Bc - W:]  # [512, W]
                blocks.append(sl.reshape(4, 128, W).transpose(1, 0, 2)
                              .reshape(128, 4 * W))
        xpk = np.ascontiguousarray(np.concatenate(blocks, axis=1))
        mzf = np.stack([(WSCALE * BIG * (pos < Bc - cnt[c])).astype(f32)
                        for cnt in f_cnts])
        mzb = np.stack([(WSCALE * BIG * (pos < Bc - cnt[c])).astype(f32)
                        for cnt in b_cnts])
        in_maps.append({
            "xpk": xpk, "wf": wf, "wb": wb, "wf8": wf8, "wb8": wb8,
            "w1": w1, "w2": w2,
            "bias": bias, "maskzf": mzf, "maskzb": mzb,
        })

    trace = bool(os.environ.get("GRU_TRACE"))
    kw = {}
    if os.environ.get("GRU_TMPDIR"):
        kw["tmpdir"] = os.environ["GRU_TMPDIR"]
    res = run_bass_kernel_spmd(nc, in_maps, core_ids=list(range(NCORES)),
                               trace=trace, **kw)
    global LAST_RESULT
    LAST_RESULT = res
    out = np.empty((B, H), f32)
    for c in range(NCORES):
        out[order[c::NCORES]] = res.results[c]["yT"].T
    return out


# revision 24
# speedup vs baseline: 1.3265x; 1.0121x over previous
"""BiGRU encoder kernel for 8 Trainium2 NeuronCores (fp16, exact ragged schedule).

Strategy:
  - Masked fixed-position reformulation: forward runs positions ascending into
    the center, backward descending into the center, so every sample's
    recurrence ENDS on the final step.  A sample of length l starts updating
    at the step where need == l; before that its hidden state is held at 0
    exactly by forcing z = 1 (+BIG on the z pre-activation).
  - Samples sorted by window_len, dealt round-robin to 8 cores (data
    parallel).  Each core holds ONE sorted batch of 1024 columns
    (features on SBUF partitions, samples on the free dim).  Step j runs on
    the exact suffix W_j = max over cores of #samples with len >= need --
    fp16 matmuls are full rate at any width, so no minimum-width padding.
  - Within a step, columns split into F (samples taking their first step:
    h == 0) and R (already running).  The hidden projection runs ONLY on R;
    F columns take a cheap h'=(1-z)n update that never reads h_prev.
    Cross-core width slack is fixed up by a narrow mask strip on z.
  - The suffix splits into 512-wide groups (PSUM bank limit).  Gate biases
    are folded into scalar_tensor_tensor ops so tanh and the h-update run as
    single wide ops over [128, 4, W].
  - Output is written feature-major (yT) and transposed on the host.
"""

import os
from contextlib import ExitStack

import numpy as np

import concourse.bacc as bacc
import concourse.tile as tile
from concourse import mybir
from concourse.bass_utils import run_bass_kernel_spmd

NCORES = 8
B, T, D, H = 8192, 15, 512, 512
G = 3 * H
Bc = B // NCORES  # 1024 columns per core
BIG = 40.0
F32 = mybir.dt.float32
DT = mybir.dt.float16
DT8 = mybir.dt.float8e4
DR = mybir.MatmulPerfMode.DoubleRow
WSCALE = 32.0

ACT = mybir.ActivationFunctionType
ALU = mybir.AluOpType

_PROGRAM_CACHE = {}
LAST_RESULT = None


def _ceil(a, b):
    return -(-a // b)


def _build_program(sched):
    """sched = (f_steps, b_steps); each steps = tuple of (W, strip) per step,
    W monotone nondecreasing, W[-1] == Bc."""
    f_steps, b_steps = sched
    nf, nb = len(f_steps), len(b_steps)

    nc = bacc.Bacc("TRN2", target_bir_lowering=False, debug=False,
                   num_devices=NCORES)

    S4 = 4 * (sum(w for w, _ in f_steps) + sum(w for w, _ in b_steps))
    xpk_d = nc.dram_tensor("xpk", [128, S4], DT, kind="ExternalInput")
    xpk8_d = nc.dram_tensor("xpk8", [128, S4], DT8, kind="ExternalInput")
    wf_d = nc.dram_tensor("wf", [D, G], DT, kind="ExternalInput")
    wb_d = nc.dram_tensor("wb", [D, G], DT, kind="ExternalInput")
    wf8_d = nc.dram_tensor("wf8", [H, G], DT8, kind="ExternalInput")
    wb8_d = nc.dram_tensor("wb8", [H, G], DT8, kind="ExternalInput")
    wf8x_d = nc.dram_tensor("wf8x", [D, 2 * H], DT8, kind="ExternalInput")
    wb8x_d = nc.dram_tensor("wb8x", [D, 2 * H], DT8, kind="ExternalInput")
    w1_d = nc.dram_tensor("w1", [2 * H, H], DT, kind="ExternalInput")
    w2_d = nc.dram_tensor("w2", [H, H], DT, kind="ExternalInput")
    bias_d = nc.dram_tensor("bias", [40, 128], F32, kind="ExternalInput")
    mf_d = nc.dram_tensor("maskzf", [nf, Bc], F32, kind="ExternalInput")
    mb_d = nc.dram_tensor("maskzb", [nb, Bc], F32, kind="ExternalInput")
    y_d = nc.dram_tensor("yT", [H, Bc], DT, kind="ExternalOutput")

    with tile.TileContext(nc) as tc, ExitStack() as ctx:
        const = ctx.enter_context(tc.tile_pool(name="const", bufs=1))
        wpool = ctx.enter_context(tc.tile_pool(name="w", bufs=2))
        w8pool = ctx.enter_context(tc.tile_pool(name="w8", bufs=2))
        w8xpool = ctx.enter_context(tc.tile_pool(name="w8x", bufs=2))
        h8fp = ctx.enter_context(tc.tile_pool(name="h8f", bufs=1))
        h8bp = ctx.enter_context(tc.tile_pool(name="h8b", bufs=1))
        xf8p = ctx.enter_context(tc.tile_pool(name="xf8", bufs=2))
        xb8p = ctx.enter_context(tc.tile_pool(name="xb8", bufs=1))
        xfp = ctx.enter_context(tc.tile_pool(name="xf", bufs=2))
        xbp = ctx.enter_context(tc.tile_pool(name="xb", bufs=2))
        hfp = ctx.enter_context(tc.tile_pool(name="hf", bufs=2))
        hbp = ctx.enter_context(tc.tile_pool(name="hb", bufs=2))
        hfin = ctx.enter_context(tc.tile_pool(name="hfin", bufs=2))
        rp = ctx.enter_context(tc.tile_pool(name="rp", bufs=2))
        z4p = ctx.enter_context(tc.tile_pool(name="z4", bufs=2))
        ssp = ctx.enter_context(tc.tile_pool(name="ss", bufs=2))
        scr = ctx.enter_context(tc.tile_pool(name="scr", bufs=2))
        ttp = ctx.enter_context(tc.tile_pool(name="tt", bufs=2))
        obuf = ctx.enter_context(tc.tile_pool(name="o", bufs=2))
        mpool = ctx.enter_context(tc.tile_pool(name="m", bufs=2))
        accp = ctx.enter_context(tc.tile_pool(name="mlp", bufs=2))
        rzps = ctx.enter_context(tc.tile_pool(name="rz", bufs=2, space="PSUM"))
        xpps = ctx.enter_context(tc.tile_pool(name="xp", bufs=2, space="PSUM"))
        ghps = ctx.enter_context(tc.tile_pool(name="gh", bufs=2, space="PSUM"))

        # Weights as [128, kchunk, gate-cols]; kchunks 0-3 input dims, 4-7
        # hidden dims.  Per-kchunk DMAs so the first matmuls start as soon
        # as chunk 0 lands.
        def load_w(dram, kchunks, cols, name, pool, tag, eng):
            t_ = pool.tile([128, kchunks, cols], DT, tag=tag, name=name)
            src = dram.rearrange("(c k) g -> k c g", k=128)
            for c in range(kchunks):
                eng.dma_start(t_[:, c, :], src[:, c, :])
            return t_

        bt = const.tile([128, 40], F32)
        nc.gpsimd.dma_start(bt[:], bias_d.rearrange("n p -> p n"))
        wf = load_w(wf_d, 4, G, "wf", wpool, "w", nc.scalar)
        wb = load_w(wb_d, 4, G, "wb", wpool, "w", nc.gpsimd)
        wf8 = w8pool.tile([128, 4, G], DT8, tag="w8", name="wf8")
        wb8 = w8pool.tile([128, 4, G], DT8, tag="w8", name="wb8")
        for c in range(4):
            nc.scalar.dma_start(
                wf8[:, c, :], wf8_d.rearrange("(c k) g -> k c g", k=128)[:, c, :])
        for c in range(4):
            nc.gpsimd.dma_start(
                wb8[:, c, :], wb8_d.rearrange("(c k) g -> k c g", k=128)[:, c, :])
        wf8x = w8xpool.tile([128, 4, 2 * H], DT8, tag="w8x", name="wf8x")
        wb8x = w8xpool.tile([128, 4, 2 * H], DT8, tag="w8x", name="wb8x")
        for c in range(4):
            nc.scalar.dma_start(
                wf8x[:, c, :], wf8x_d.rearrange("(c k) g -> k c g", k=128)[:, c, :])
        for c in range(4):
            nc.gpsimd.dma_start(
                wb8x[:, c, :], wb8x_d.rearrange("(c k) g -> k c g", k=128)[:, c, :])

        def emit_x(steps, pool, tag, j, off):
            # x tile holds the step's suffix packed [4k x W] contiguously per
            # partition -- single fat DMA run on both sides.
            W = steps[j][0]
            xt = pool.tile([128, 4 * Bc], DT, tag=tag, name=f"{tag}{j}")
            nc.sync.dma_start(xt[:, :4 * W], xpk_d[:, off:off + 4 * W])
            return xt

        def emit_x8(steps, pool, tag, j, off):
            W = steps[j][0]
            xt = pool.tile([128, 4 * Bc], DT8, tag=tag, name=f"{tag}{j}")
            nc.sync.dma_start(xt[:, :4 * W], xpk8_d[:, off:off + 4 * W])
            return xt

        def emit_step(j, steps, xt, x8t, hh_prev, w, w8, w8x, mask_d, bb,
                      hpool, h8pool, tag, is_last):
            """One GRU step.  Local cols 0..W-1 map to global Bc-W..Bc-1.
            F = [0, Fw): first-step columns.  R = [Fw, W): running."""
            W, strip = steps[j]
            Wprev = steps[j - 1][0] if j > 0 else 0
            Fw = W - Wprev
            goff = Bc - W  # local -> global
            h_prev, h8_prev = hh_prev if hh_prev else (None, None)

            h_next = (hfin if is_last else hpool).tile(
                [128, 4, Bc], DT, tag="hfin" if is_last else tag, name="h")
            h8_next = None
            if not is_last:
                h8_next = h8pool.tile([128, 4, Bc], DT8, tag=tag + "8",
                                      name="h8")

            mt = None
            if strip > 0:
                mt = mpool.tile([128, 64], F32, tag="m", name="mt")
                nc.sync.dma_start(
                    mt[:, :strip],
                    mask_d[j, goff:goff + strip].partition_broadcast(128),
                )

            ngroups = _ceil(W, 512)
            for g in range(ngroups):  # left-aligned groups on local coords
                glo, ghi = 512 * g, min(512 * (g + 1), W)
                gw = ghi - glo
                fl, fh = glo, max(min(ghi, Fw), glo)   # F within group
                rl, rh = max(glo, Fw), ghi             # R within group
                fw, rw = fh - fl, rh - rl
                sl, sh = glo, max(min(ghi, strip), glo)  # mask strip in group
                sw = sh - sl

                rz = []
                xpn = []
                ghn = []
                x8v = None
                if x8t is not None:
                    x8v = x8t[:, :4 * W].rearrange("p (k w) -> p k w", k=4)
                for i in range(4):
                    ro, zo, no = i * 128, H + i * 128, 2 * H + i * 128
                    ro8, zo8 = i * 128, H + i * 128
                    rzt = rzps.tile([128, 2, 512], F32, tag="rz", name=f"rz{i}")
                    xpt = xpps.tile([128, 512], F32, tag="xp", name=f"xp{i}")
                    rz.append(rzt)
                    xpn.append(xpt)
                    if x8v is not None:
                        # r/z x-proj in fp8 DoubleRow (2 k-chunks per pass)
                        for p in range(2):
                            st = p == 0
                            ks = slice(2 * p, 2 * p + 2)
                            if fw > 0 and rw > 0:
                                xF = x8v[:, ks, fl:fh]
                                xR = x8v[:, ks, rl:rh]
                                nc.tensor.matmul(rzt[:, 0, :fw],
                                                 w8x[:, ks, ro8:ro8 + 128], xF,
                                                 start=st, stop=p == 1,
                                                 perf_mode=DR)
                                nc.tensor.matmul(rzt[:, 0, fw:gw],
                                                 w8x[:, ks, ro8:ro8 + 128], xR,
                                                 start=False, stop=False,
                                                 perf_mode=DR,
                                                 skip_group_check=True)
                                nc.tensor.matmul(rzt[:, 1, :fw],
                                                 w8x[:, ks, zo8:zo8 + 128], xF,
                                                 start=st, stop=p == 1,
                                                 perf_mode=DR)
                                nc.tensor.matmul(rzt[:, 1, fw:gw],
                                                 w8x[:, ks, zo8:zo8 + 128], xR,
                                                 start=False, stop=False,
                                                 perf_mode=DR,
                                                 skip_group_check=True)
                            else:
                                sp = p == 1 and rw == 0
                                xg = x8v[:, ks, glo:ghi]
                                nc.tensor.matmul(rzt[:, 0, :gw],
                                                 w8x[:, ks, ro8:ro8 + 128], xg,
                                                 start=st, stop=sp,
                                                 perf_mode=DR)
                                nc.tensor.matmul(rzt[:, 1, :gw],
                                                 w8x[:, ks, zo8:zo8 + 128], xg,
                                                 start=st, stop=sp,
                                                 perf_mode=DR)
                    else:
                        for k in range(4):
                            st = k == 0
                            xk = xt[:, k * W + glo:k * W + ghi]
                            if fw > 0 and rw > 0:
                                xkF = xt[:, k * W + fl:k * W + fh]
                                xkR = xt[:, k * W + rl:k * W + rh]
                                nc.tensor.matmul(rzt[:, 0, :fw],
                                                 w[:, k, ro:ro + 128],
                                                 xkF, start=st, stop=False)
                                nc.tensor.matmul(rzt[:, 0, fw:gw],
                                                 w[:, k, ro:ro + 128],
                                                 xkR, start=False, stop=False,
                                                 skip_group_check=True)
                                nc.tensor.matmul(rzt[:, 1, :fw],
                                                 w[:, k, zo:zo + 128],
                                                 xkF, start=st, stop=False)
                                nc.tensor.matmul(rzt[:, 1, fw:gw],
                                                 w[:, k, zo:zo + 128],
                                                 xkR, start=False, stop=False,
                                                 skip_group_check=True)
                            else:
                                sp = k == 3 and rw == 0
                                nc.tensor.matmul(rzt[:, 0, :gw],
                                                 w[:, k, ro:ro + 128],
                                                 xk, start=st, stop=sp)
                                nc.tensor.matmul(rzt[:, 1, :gw],
                                                 w[:, k, zo:zo + 128],
                                                 xk, start=st, stop=sp)
                    for k in range(4):
                        xk = xt[:, k * W + glo:k * W + ghi]
                        nc.tensor.matmul(xpt[:, :gw], w[:, k, no:no + 128],
                                         xk, start=k == 0, stop=k == 3)
                    if rw > 0:
                        ght = ghps.tile([128, 512], F32, tag="gh", name=f"gh{i}")
                        ghn.append(ght)
                        fo = rl - glo  # local-in-group offset of R
                        for p in range(2):  # fp8 DoubleRow: 2 k-chunks/pass
                            hk = h8_prev[:, 2 * p:2 * p + 2,
                                         goff + rl:goff + rh]
                            nc.tensor.matmul(rzt[:, 0, fo:gw],
                                             w8[:, 2 * p:2 * p + 2, ro:ro + 128],
                                             hk, start=False, stop=p == 1,
                                             perf_mode=DR)
                            nc.tensor.matmul(rzt[:, 1, fo:gw],
                                             w8[:, 2 * p:2 * p + 2, zo:zo + 128],
                                             hk, start=False, stop=p == 1,
                                             perf_mode=DR)
                            nc.tensor.matmul(ght[:, :rw],
                                             w8[:, 2 * p:2 * p + 2, no:no + 128],
                                             hk, start=p == 0, stop=p == 1,
                                             perf_mode=DR)

                # --- elementwise chain for this group, per i-pair so the
                # next step's h-projection k-chunks can start early ---
                z4 = z4p.tile([128, 4, 512], DT, tag="z4", name="z4")
                ss4 = ssp.tile([128, 4, 512], DT, tag="ss", name="ss4")
                sc = scr.tile([128, 4, 512], DT, tag="scr", name="sc")
                rts = []

                def upd_pair(p0):
                    ii = slice(p0, p0 + 2)
                    # tanh in place: ss4 becomes n
                    nc.scalar.activation(ss4[:, ii, :gw], ss4[:, ii, :gw],
                                         ACT.Tanh, scale=1.0 / WSCALE)
                    if fw > 0:
                        zF = z4[:, ii, fl - glo:fh - glo]
                        nF = ss4[:, ii, fl - glo:fh - glo]
                        eF = sc[:, ii, fl - glo:fh - glo]
                        nc.vector.tensor_mul(eF, zF, nF)
                        nc.vector.tensor_sub(h_next[:, ii, goff + fl:goff + fh],
                                             nF, eF)
                    if rw > 0:
                        zR = z4[:, ii, rl - glo:rh - glo]
                        nR = ss4[:, ii, rl - glo:rh - glo]
                        dd = sc[:, ii, rl - glo:rh - glo]
                        nc.vector.tensor_sub(dd, h_prev[:, ii, goff + rl:goff + rh],
                                             nR)
                        nc.vector.tensor_mul(dd, zR, dd)
                        nc.vector.tensor_add(h_next[:, ii, goff + rl:goff + rh],
                                             nR, dd)
                    if h8_next is not None:
                        nc.vector.tensor_copy(
                            h8_next[:, ii, goff + glo:goff + ghi],
                            h_next[:, ii, goff + glo:goff + ghi])

                for i in range(4):
                    # r = sigmoid(rps + bias_r)
                    rt = rp.tile([128, 512], DT, tag="r", name=f"rt{i}")
                    rts.append(rt)
                    nc.scalar.activation(rt[:, :gw], rz[i][:, 0, :gw],
                                         ACT.Sigmoid, bias=bt[:, bb + i:bb + i + 1],
                                         scale=1.0 / WSCALE)
                    # mask strip: force z -> 1 on over-included columns
                    if sw > 0:
                        nc.vector.tensor_add(rz[i][:, 1, sl - glo:sh - glo],
                                             rz[i][:, 1, sl - glo:sh - glo],
                                             mt[:, sl:sh])
                    nc.scalar.activation(z4[:, i, :gw], rz[i][:, 1, :gw],
                                         ACT.Sigmoid,
                                         bias=bt[:, bb + 4 + i:bb + 5 + i],
                                         scale=1.0 / WSCALE)
                    # n pre-activation: ss = xpn + bih_n + r * (ghn + bhh_n)
                    if fw > 0:
                        t1 = ttp.tile([128, 512], DT, tag="tt", name="t1")
                        nc.vector.tensor_scalar(
                            t1[:, :fw], rt[:, fl - glo:fh - glo],
                            bt[:, bb + 8 + i:bb + 9 + i],
                            bt[:, bb + 12 + i:bb + 13 + i],
                            op0=ALU.mult, op1=ALU.add)
                        nc.vector.tensor_add(
                            ss4[:, i, fl - glo:fh - glo], t1[:, :fw],
                            xpn[i][:, fl - glo:fh - glo])
                    if rw > 0:
                        t2 = ttp.tile([128, 512], DT, tag="tt", name="t2")
                        nc.vector.scalar_tensor_tensor(
                            t2[:, :rw], ghn[i][:, :rw],
                            bt[:, bb + 8 + i:bb + 9 + i],
                            rt[:, rl - glo:rh - glo],
                            op0=ALU.add, op1=ALU.mult)
                        nc.vector.scalar_tensor_tensor(
                            ss4[:, i, rl - glo:rh - glo], t2[:, :rw],
                            bt[:, bb + 12 + i:bb + 13 + i],
                            xpn[i][:, rl - glo:rh - glo],
                            op0=ALU.add, op1=ALU.add)
                    if i == 1:
                        upd_pair(0)
                    elif i == 3:
                        upd_pair(2)
            return h_next, h8_next

        # Interleave the two directions step-by-step: while one direction's
        # elementwise chain drains, the PE runs the other direction's
        # (independent) matmuls -- kills recurrence-latency bubbles at the
        # narrow early steps.
        hhf = hhb = None
        off_f, off_b = 0, 4 * sum(w for w, _ in f_steps)
        for j in range(max(nf, nb)):
            if j < nf:
                xt = emit_x(f_steps, xfp, "xf", j, off_f)
                x8t = (emit_x8(f_steps, xf8p, "xf8", j, off_f)
                       if j < nf - 1 else None)
                off_f += 4 * f_steps[j][0]
                hhf = emit_step(j, f_steps, xt, x8t, hhf, wf, wf8, wf8x,
                                mf_d, 0, hfp, h8fp, "hf", j == nf - 1)
            if j < nb:
                xt = emit_x(b_steps, xbp, "xb", j, off_b)
                x8t = (emit_x8(b_steps, xb8p, "xb8", j, off_b)
                       if j < nb - 1 else None)
                off_b += 4 * b_steps[j][0]
                hhb = emit_step(j, b_steps, xt, x8t, hhb, wb, wb8, wb8x,
                                mb_d, 16, hbp, h8bp, "hb", j == nb - 1)
        hf4, hb4 = hhf[0], hhb[0]

        # w1/w2 load mid-kernel: keeps the HBM-bound warmup window clear
        w1 = load_w(w1_d, 8, H, "w1", const, "w1", nc.gpsimd)
        w2 = load_w(w2_d, 4, H, "w2", const, "w2", nc.gpsimd)

        # MLP phase A: acc = W1[:, :H].T @ hf -- independent of hb, fills the
        # PE while the last backward step's chain drains.  Uses the ghps PSUM
        # pool (freed early in the GRU chain) to avoid stalling on the last
        # backward step's xpn consumers.
        acc = accp.tile([128, 4, Bc], DT, tag="mlp", name="acc")
        for g in range(Bc // 512):
            for i in range(4):
                ps = ghps.tile([128, 512], F32, tag="gh", name="mlpA")
                for k in range(4):
                    nc.tensor.matmul(ps[:], w1[:, k, i * 128:(i + 1) * 128],
                                     hf4[:, k, g * 512:(g + 1) * 512],
                                     start=k == 0, stop=k == 3)
                nc.scalar.activation(acc[:, i, g * 512:(g + 1) * 512], ps[:],
                                     ACT.Copy)

        # MLP phase B: hid = relu(acc + W1[:, H:].T @ hb + b1)
        hid = accp.tile([128, 4, Bc], DT, tag="mlp", name="hid")
        for g in range(Bc // 512):
            gs = slice(g * 512, (g + 1) * 512)
            pre = ssp.tile([128, 4, 512], DT, tag="ss", name="pre")
            for i in range(4):
                ps = xpps.tile([128, 512], F32, tag="xp", name="mlpB")
                for k in range(4):
                    nc.tensor.matmul(ps[:], w1[:, 4 + k, i * 128:(i + 1) * 128],
                                     hb4[:, k, gs], start=k == 0, stop=k == 3)
                nc.vector.scalar_tensor_tensor(
                    pre[:, i, :], ps[:], bt[:, 32 + i:33 + i],
                    acc[:, i, gs], op0=ALU.add, op1=ALU.add)
            nc.scalar.activation(hid[:, :, gs], pre[:], ACT.Relu)
        # MLP phase C: y = W2.T @ hid + b2, written feature-major fp16
        for g in range(Bc // 512):
            gs = slice(g * 512, (g + 1) * 512)
            for i in range(4):
                ps = ghps.tile([128, 512], F32, tag="gh", name="mlpC")
                for k in range(4):
                    nc.tensor.matmul(ps[:], w2[:, k, i * 128:(i + 1) * 128],
                                     hid[:, k, gs], start=k == 0, stop=k == 3)
                o32 = obuf.tile([128, 512], DT, tag="o", name="o32")
                nc.scalar.activation(o32[:], ps[:], ACT.Identity,
                                     bias=bt[:, 36 + i:37 + i])
                nc.scalar.dma_start(
                    y_d[i * 128:(i + 1) * 128, gs], o32[:])

    nc.compile()
    return nc


def kernel(padded_window, window_len, Wih_f, Whh_f, bih_f, bhh_f,
           Wih_b, Whh_b, bih_b, bhh_b, W1, b1, W2, b2):
    wl = np.asarray(window_len)
    lf = (wl - 1) // 2 + 1
    lb = wl // 2 + 1
    order = np.argsort(wl, kind="stable")

    # per-core sorted lengths: row k = per-core rank k, column = core
    lf_pc = lf[order].reshape(-1, NCORES)
    lb_pc = lb[order].reshape(-1, NCORES)

    def dir_steps(lens_pc):
        n = int(lens_pc.max())
        steps, cnts = [], []
        for j in range(n):
            need = n - j
            cnt = (lens_pc >= need).sum(axis=0)  # per core
            W = int(cnt.max())
            strip = W - int(cnt.min())
            assert strip <= 64, f"mask strip {strip} exceeds tile"
            steps.append((W, strip))
            cnts.append(cnt)
        return tuple(steps), cnts

    f_steps, f_cnts = dir_steps(lf_pc)
    b_steps, b_cnts = dir_steps(lb_pc)
    sched = (f_steps, b_steps)

    if sched not in _PROGRAM_CACHE:
        _PROGRAM_CACHE[sched] = _build_program(sched)
    nc = _PROGRAM_CACHE[sched]

    import ml_dtypes
    f16 = np.float16
    f32 = np.float32
    f8 = ml_dtypes.float8_e4m3
    wf = np.ascontiguousarray(Wih_f.T * WSCALE, dtype=f16)
    wb = np.ascontiguousarray(Wih_b.T * WSCALE, dtype=f16)
    wf8 = np.ascontiguousarray((Whh_f.T * WSCALE).astype(f8))
    wb8 = np.ascontiguousarray((Whh_b.T * WSCALE).astype(f8))
    wf8x = np.ascontiguousarray((Wih_f.T[:, :2 * H] * WSCALE).astype(f8))
    wb8x = np.ascontiguousarray((Wih_b.T[:, :2 * H] * WSCALE).astype(f8))
    w1 = np.ascontiguousarray(W1.T, dtype=f16)
    w2 = np.ascontiguousarray(W2.T, dtype=f16)

    def chunks(v):  # [512] -> [4, 128]
        return np.asarray(v, f32).reshape(4, 128)

    bias = np.concatenate([
        chunks((bih_f + bhh_f)[:H]), chunks((bih_f + bhh_f)[H:2 * H]),
        chunks(WSCALE * bhh_f[2 * H:]), chunks(WSCALE * bih_f[2 * H:]),
        chunks((bih_b + bhh_b)[:H]), chunks((bih_b + bhh_b)[H:2 * H]),
        chunks(WSCALE * bhh_b[2 * H:]), chunks(WSCALE * bih_b[2 * H:]),
        chunks(b1), chunks(b2),
    ], 0)  # [40, 128]

    pw16 = np.asarray(padded_window).astype(f16)
    pos = np.arange(Bc)
    nf, nb = len(f_steps), len(b_steps)
    in_maps = []
    for c in range(NCORES):
        idx = order[c::NCORES]
        xTc = pw16[idx].transpose(1, 2, 0)  # [15, 512, Bc] (view-ish)
        blocks = []
        for steps, pfn in ((f_steps, lambda j: 8 - nf + j),
                           (b_steps, lambda j: 6 + nb - j)):
            for j, (W, _) in enumerate(steps):
                sl = xTc[pfn(j), :, Bc - W:]  # [512, W]
                blocks.append(sl.reshape(4, 128, W).transpose(1, 0, 2)
                              .reshape(128, 4 * W))
        xpk = np.ascontiguousarray(np.concatenate(blocks, axis=1))
        xpk8 = np.ascontiguousarray(xpk.astype(f8))
        mzf = np.stack([(WSCALE * BIG * (pos < Bc - cnt[c])).astype(f32)
                        for cnt in f_cnts])
        mzb = np.stack([(WSCALE * BIG * (pos < Bc - cnt[c])).astype(f32)
                        for cnt in b_cnts])
        in_maps.append({
            "xpk": xpk, "xpk8": xpk8, "wf": wf, "wb": wb,
            "wf8": wf8, "wb8": wb8, "wf8x": wf8x, "wb8x": wb8x,
            "w1": w1, "w2": w2,
            "bias": bias, "maskzf": mzf, "maskzb": mzb,
        })

    trace = bool(os.environ.get("GRU_TRACE"))
    kw = {}
    if os.environ.get("GRU_TMPDIR"):
        kw["tmpdir"] = os.environ["GRU_TMPDIR"]
    res = run_bass_kernel_spmd(nc, in_maps, core_ids=list(range(NCORES)),
                               trace=trace, **kw)
    global LAST_RESULT
    LAST_RESULT = res
    out = np.empty((B, H), f32)
    for c in range(NCORES):
        out[order[c::NCORES]] = res.results[c]["yT"].T
    return out
